# revision 1
# baseline (speedup 1.0000x reference)
"""Single-head causal attention block (QKV projection + attention) on 8 TRN2 cores.

Reference computation (per batch element b, batch-sharded 1 core each):
    qkv = x[b] @ W.T + b          # [T, 3E]
    q, k, v = split(qkv)          # each [T, E]
    s = (q @ k.T) / sqrt(E), causal-masked
    y = softmax(s) @ v            # [T, E]

Shapes: B=8, T=2048, E=1024.  Design notes:
  - All layouts host-prepped so no on-device transposes are needed:
      q^T, k^T computed in [E, T] layout (score matmul operands),
      v computed in [T, E] layout (PV matmul rhs),
      scores computed transposed S^T[tk, tq] so exp needs no partition reduce.
  - Softmax without max-subtraction: scores for this problem are ~N(0, 0.33),
    |s| < 3, so exp() is numerically safe unnormalized; masked entries get -50
    added (exp -> ~2e-22).  Row sums Z come from a ones-column matmul fused
    into the PV accumulation pattern; normalization is a per-partition
    tensor_scalar multiply at the end.
  - Causal structure skips entire 128x512 score tiles above the diagonal and
    the corresponding PV accumulation terms (~2x on attention FLOPs).
"""

import numpy as np
import ml_dtypes
from contextlib import ExitStack

import concourse.bass as bass
import concourse.bacc as bacc
import concourse.mybir as mybir
import concourse.tile as tile
from concourse.bass_utils import run_bass_kernel_spmd

FP32 = mybir.dt.float32
F32R = mybir.dt.float32r
BF16 = mybir.dt.bfloat16
AF = mybir.ActivationFunctionType

B, T, E = 8, 2048, 1024
P = 128
NE = E // P            # 8 e-tiles (contraction)
NT = T // P            # 16 t-tiles
NC = 4                 # tq chunks of 512
CH = T // NC           # 512
SCALE = 1.0 / np.sqrt(E)
MASK_NEG = -50.0

# dtype knobs
P1_DT = BF16           # phase-1 matmul operand dtype (x, W)
QK_DT = BF16           # stored q^T / k^T dtype (score matmul operands)
V_DT = F32R            # stored v dtype (fp32r: full-rate matmul, ~fp32 mantissa)
ES_DT = F32R           # stored exp(S^T) dtype (fp32r)


def _np_of(dt):
    return ml_dtypes.bfloat16 if dt == BF16 else np.float32


def _build_nc(n_reps=1):
    nc = bacc.Bacc()

    xt_d = nc.declare_dram_parameter("xt", [NE, P, T], P1_DT, isOutput=False)
    wqk_d = nc.declare_dram_parameter("wqk", [2 * NE, P, NE, P], P1_DT, isOutput=False)
    wv_d = nc.declare_dram_parameter("wv", [P, NE, E], P1_DT, isOutput=False)
    bqk_d = nc.declare_dram_parameter("bqk", [2 * NE, P, 1], FP32, isOutput=False)
    bvrep_d = nc.declare_dram_parameter("bvrep", [P, E], FP32, isOutput=False)
    masks_d = nc.declare_dram_parameter("masks", [4, P, CH], FP32, isOutput=False)
    onesc_d = nc.declare_dram_parameter("onesc", [P, 4], F32R, isOutput=False)
    y_d = nc.declare_dram_parameter("y", [T, E], FP32, isOutput=True)

    with tile.TileContext(nc) as tc:
        with ExitStack() as ctx:
            # ---- persistent pools (live through whole kernel) ----
            const_pool = ctx.enter_context(tc.tile_pool(name="const", bufs=1))
            bqk_pool = ctx.enter_context(tc.tile_pool(name="bqk", bufs=2 * NE))
            mask_pool = ctx.enter_context(tc.tile_pool(name="mask", bufs=4))
            qk_pool = ctx.enter_context(tc.tile_pool(name="qk", bufs=2 * NE))
            v_pool = ctx.enter_context(tc.tile_pool(name="v", bufs=NT))
            psa = ctx.enter_context(tc.tile_pool(name="psa", bufs=3, space="PSUM"))

            ones_col = const_pool.tile([P, 4], F32R, tag="ones", name="ones")
            nc.sync.dma_start(ones_col[:], onesc_d[:])

            qk_sb = [qk_pool.tile([P, T], QK_DT, tag="qk", name="qk") for _ in range(2 * NE)]
            v_sb = [v_pool.tile([P, E], V_DT, tag="v", name="v") for _ in range(NT)]

            # benchmark-only: run the whole body n_reps times on-device so
            # per-kernel time can be extracted from wall-clock deltas
            rep_ctx = tc.For_i(0, n_reps, 1) if n_reps > 1 else None
            if rep_ctx is not None:
                ctx.enter_context(rep_ctx)

            # ---- phase 1: qkv projection ----
            with ExitStack() as p1:
                xt_pool = p1.enter_context(tc.tile_pool(name="xt", bufs=NE))
                wqk_pool = p1.enter_context(tc.tile_pool(name="wqkp", bufs=4))
                wv_pool = p1.enter_context(tc.tile_pool(name="wvp", bufs=1))

                # xt first (critical path: every phase-1 matmul group needs
                # all 8 e-tiles); weights go on the scalar-engine HWDGE
                # queue so they stream in parallel with xt on the sync queue
                xt_sb = []
                for a in range(NE):
                    t_ = xt_pool.tile([P, T], P1_DT, tag="xt", name="xt")
                    nc.sync.dma_start(t_[:], xt_d[a])
                    xt_sb.append(t_)
                bqk_sb = []
                for ft in range(2 * NE):
                    t_ = bqk_pool.tile([P, 1], FP32, tag="bqk", name="bqk")
                    nc.sync.dma_start(t_[:], bqk_d[ft])
                    bqk_sb.append(t_)

                # q^T and k^T in [f, t] layout, f-tile by f-tile
                for ft in range(2 * NE):
                    wt = wqk_pool.tile([P, NE, P], P1_DT, tag="wqk", name="wqk")
                    # split load per e-slice: a single big DMA fans out over
                    # two HW queues, and a slot-reuse DMA can then need 3 sync
                    # waits (PE release + 2x WAW) -- more than the DIRECT2D
                    # encoding's 2 slots.  Small DMAs stay single-queue.
                    for e in range(NE):
                        nc.scalar.dma_start(wt[:, e, :], wqk_d[ft, :, e, :])
                    if ft == 2:
                        # weights for the v path arrive while qk streams
                        wv_sb = wv_pool.tile([P, NE, E], P1_DT, tag="wv", name="wv")
                        for e in range(NE):
                            nc.scalar.dma_start(wv_sb[:, e, :], wv_d[:, e, :])
                        bvrep = const_pool.tile([P, E], FP32, tag="bvrep", name="bvrep")
                        nc.sync.dma_start(bvrep[:], bvrep_d[:])
                    for tch in range(NC):
                        ps = psa.tile([P, CH], FP32, tag="ps", name="ps")
                        for e in range(NE):
                            nc.tensor.matmul(
                                ps[:],
                                lhsT=wt[:, e, :],
                                rhs=xt_sb[e][:, tch * CH:(tch + 1) * CH],
                                start=(e == 0),
                                stop=(e == NE - 1),
                            )
                        # bias add + 1/sqrt(E) score scale folded into q
                        # copy-out: out = in*scale + bias (bias prescaled)
                        sc = SCALE if ft < NE else 1.0
                        nc.scalar.activation(
                            qk_sb[ft][:, tch * CH:(tch + 1) * CH],
                            ps[:],
                            AF.Identity,
                            bias=bqk_sb[ft][:],
                            scale=sc,
                        )

                # v in [t, e] layout
                for tt in range(NT):
                    for ec in range(2):
                        ps = psa.tile([P, CH], FP32, tag="ps", name="ps")
                        for e in range(NE):
                            nc.tensor.matmul(
                                ps[:],
                                lhsT=xt_sb[e][:, tt * P:(tt + 1) * P],
                                rhs=wv_sb[:, e, ec * CH:(ec + 1) * CH],
                                start=(e == 0),
                                stop=(e == NE - 1),
                            )
                        # bias varies along free dim -> tensor add of
                        # host-replicated bias tile, writes V_DT directly
                        nc.vector.tensor_add(
                            v_sb[tt][:, ec * CH:(ec + 1) * CH],
                            ps[:],
                            bvrep[:, ec * CH:(ec + 1) * CH],
                        )

            # ---- phases 2+3: scores+softmax+PV, per tq chunk ----
            with ExitStack() as p2:
                exps_pool = p2.enter_context(tc.tile_pool(name="exps", bufs=18))
                y_pool = p2.enter_context(tc.tile_pool(name="yst", bufs=3))
                zr_pool = p2.enter_context(tc.tile_pool(name="zr", bufs=4))
                psy = p2.enter_context(tc.tile_pool(name="psy", bufs=3, space="PSUM"))
                psz = p2.enter_context(tc.tile_pool(name="psz", bufs=2, space="PSUM"))

                mask_sb = []
                for d in range(4):
                    t_ = mask_pool.tile([P, CH], FP32, tag="mask", name="mask")
                    nc.sync.dma_start(t_[:], masks_d[d])
                    mask_sb.append(t_)

                for c in range(NC):
                    n_tk = (c + 1) * (CH // P)  # tk tiles at/below diagonal
                    exps_tiles = []
                    for tk in range(n_tk):
                        ps = psa.tile([P, CH], FP32, tag="ps", name="ps")
                        for e in range(NE):
                            nc.tensor.matmul(
                                ps[:],
                                lhsT=qk_sb[NE + e][:, tk * P:(tk + 1) * P],
                                rhs=qk_sb[e][:, c * CH:(c + 1) * CH],
                                start=(e == 0),
                                stop=(e == NE - 1),
                            )
                        d = tk - c * (CH // P)
                        if d >= 0:  # diagonal-crossing tile: additive causal mask
                            nc.vector.tensor_add(ps[:], ps[:], mask_sb[d][:])
                        et = exps_pool.tile([P, CH], ES_DT, tag="es", name="es")
                        nc.scalar.activation(et[:], ps[:], AF.Exp)
                        exps_tiles.append(et)

                    ps_z = psz.tile([P, 4 * (CH // P)], FP32, tag="z", name="z")
                    for j in range(CH // P):
                        tq = c * (CH // P) + j
                        nj = tq + 1
                        for tk in range(nj):
                            nc.tensor.matmul(
                                ps_z[:, 4 * j:4 * j + 4],
                                lhsT=exps_tiles[tk][:, j * P:(j + 1) * P],
                                rhs=ones_col[:],
                                start=(tk == 0),
                                stop=(tk == nj - 1),
                            )
                        zr = zr_pool.tile([P, 1], FP32, tag="zr", name="zr")
                        nc.vector.reciprocal(zr[:], ps_z[:, 4 * j:4 * j + 1])
                        y_t = y_pool.tile([P, E], FP32, tag="y", name="y")
                        for ec in range(2):
                            ps_y = psy.tile([P, CH], FP32, tag="y", name="psy")
                            for tk in range(nj):
                                nc.tensor.matmul(
                                    ps_y[:],
                                    lhsT=exps_tiles[tk][:, j * P:(j + 1) * P],
                                    rhs=v_sb[tk][:, ec * CH:(ec + 1) * CH],
                                    start=(tk == 0),
                                    stop=(tk == nj - 1),
                                )
                            nc.vector.tensor_scalar_mul(
                                y_t[:, ec * CH:(ec + 1) * CH], ps_y[:], zr[:]
                            )
                        nc.sync.dma_start(y_d[tq * P:(tq + 1) * P, :], y_t[:])
    nc.finalize()  # run the Bacc pass pipeline (wait splitting, reg alloc, ...)
    return nc


_NC_CACHE = {}


def _get_nc(n_reps=1):
    if n_reps not in _NC_CACHE:
        _NC_CACHE[n_reps] = _build_nc(n_reps)
    return _NC_CACHE[n_reps]


def _prep_inputs(x, W, b):
    p1np = _np_of(P1_DT)
    # x[b].T tiled: xt[a, p, t] = x[b, t, a*128+p]
    xt = np.ascontiguousarray(
        x.reshape(B, T, NE, P).transpose(0, 2, 3, 1)
    ).astype(p1np)
    # wqk[ft, p, a, f'] = W[ft*128+f', a*128+p]
    wqk = np.ascontiguousarray(
        W[:2 * E].reshape(2 * NE, P, NE, P).transpose(0, 3, 2, 1)
    ).astype(p1np)
    # wv[p, a, eo] = W[2E+eo, a*128+p]
    wv = np.ascontiguousarray(
        W[2 * E:].reshape(E, NE, P).transpose(2, 1, 0)
    ).astype(p1np)
    # ACT applies out = in*scale + bias, so the q bias is prescaled
    bqk = b[:2 * E].astype(np.float32).copy()
    bqk[:E] *= SCALE
    bqk = bqk.reshape(2 * NE, P, 1)
    bvrep = np.broadcast_to(b[2 * E:].astype(np.float32), (P, E)).copy()
    ii = np.arange(P)[:, None]
    jj = np.arange(CH)[None, :]
    masks = np.stack(
        [np.where(jj >= d * P + ii, 0.0, MASK_NEG) for d in range(4)]
    ).astype(np.float32)
    onesc = np.ones((P, 4), np.float32)
    shared = {"wqk": wqk, "wv": wv, "bqk": bqk, "bvrep": bvrep, "masks": masks,
              "onesc": onesc}
    return [{"xt": np.ascontiguousarray(xt[i]), **shared} for i in range(B)]


def run(x, W, b, **spmd_kwargs):
    nc = _get_nc()
    in_maps = _prep_inputs(np.asarray(x), np.asarray(W), np.asarray(b))
    res = run_bass_kernel_spmd(nc, in_maps, list(range(B)), **spmd_kwargs)
    y = np.stack([res.results[i]["y"] for i in range(B)]).astype(np.float32)
    return y, res


def kernel(x, W, b):
    y, _ = run(x, W, b)
    return y



# revision 9
# speedup vs baseline: 2.4191x; 2.4191x over previous
"""Single-head causal attention block (QKV projection + attention) on 8 TRN2 cores.

Reference computation (per batch element b, batch-sharded 1 core each):
    qkv = x[b] @ W.T + b          # [T, 3E]
    q, k, v = split(qkv)          # each [T, E]
    s = (q @ k.T) / sqrt(E), causal-masked
    y = softmax(s) @ v            # [T, E]

Shapes: B=8, T=2048, E=1024.

Speed strategy: nearly all matmul FLOPs run as fp8(e4m3) DoubleRow matmuls
(0.5 cycles/row = 2x bf16 rate).  DoubleRow contracts 256 elements per
instruction: both operands carry a pair dim in a 3D AP [128, 2, N].

Accuracy strategy (rel-err budget 2e-2, achieved ~6e-3 in numpy sim):
  - W scaled x16 before e4m3 quantization (avoids subnormals); /16 folded
    into the copy-out (q,k) or into the ones/Z scaling (v path: v stored x16,
    Z computed with ones=16, so y = ps_y * (1/(16Z)) is exact).
  - q,k stored unscaled e4m3 (std ~0.58); the 1/sqrt(E) score scale is
    applied inside the Exp activation (out = exp(in*scale)).
  - causal mask applied as an extra accumulating PE matmul: (-16*I).T @
    (96*tril-indicator) = -1536 on masked entries -> exp(scale*s - 48) -> 0
    exactly in fp8.
  - softmax row sums Z from a ones(=16)-column DoubleRow matmul; masked
    entries quantize to exactly 0 so above-diagonal pair-tiles contribute 0,
    letting every PV/Z accumulation use full pairs.
  - rows t<128 (few softmax terms -> no error averaging) are recomputed on a
    high-precision bf16/fp32r path: bf16 q0,k0,v0 projections, fp32 exp,
    fp32r PV.  This bounds the worst-case row error; rows >=128 average fp8
    noise over >=129 keys.
"""

import numpy as np
import ml_dtypes
from contextlib import ExitStack

import concourse.bass as bass
import concourse.bacc as bacc
import concourse.mybir as mybir
import concourse.tile as tile
from concourse.bass_utils import run_bass_kernel_spmd

FP32 = mybir.dt.float32
F32R = mybir.dt.float32r
BF16 = mybir.dt.bfloat16
FP8 = mybir.dt.float8e4
AF = mybir.ActivationFunctionType
ALU = mybir.AluOpType
DR = mybir.MatmulPerfMode.DoubleRow

B, T, E = 8, 2048, 1024
P = 128
NE = E // P            # 8 e-tiles (contraction)
NA = NE // 2           # 4 e-pair tiles for DoubleRow
NT = T // P            # 16 t-tiles
NC = 4                 # tq chunks of 512
CH = T // NC           # 512
SCALE = 1.0 / np.sqrt(E)
WS = 16.0              # weight pre-scale before fp8 quantization
MA = -16.0             # mask matmul stationary value ...
MB = 96.0              # ... times moving value = -1536 on masked entries

E4NP = ml_dtypes.float8_e4m3fn
BFNP = ml_dtypes.bfloat16


def _build_nc():
    nc = bacc.Bacc()

    xt8_d = nc.declare_dram_parameter("xt8", [NA, P, 2, T], FP8, isOutput=False)
    xtbf_d = nc.declare_dram_parameter("xtbf", [P, NE, P], BF16, isOutput=False)
    wqk8_d = nc.declare_dram_parameter("wqk8", [2 * NE, P, NA, 2, P], FP8, isOutput=False)
    wqkbf_d = nc.declare_dram_parameter("wqkbf", [2 * NE, P, NE, P], BF16, isOutput=False)
    wv8_d = nc.declare_dram_parameter("wv8", [NA, P, 2, E], FP8, isOutput=False)
    wvbf_d = nc.declare_dram_parameter("wvbf", [P, NE, E], BF16, isOutput=False)
    bqk_d = nc.declare_dram_parameter("bqk", [2 * NE, P, 1], FP32, isOutput=False)
    bvrep_d = nc.declare_dram_parameter("bvrep", [P, E], FP32, isOutput=False)
    mska8_d = nc.declare_dram_parameter("mska8", [P, P], FP8, isOutput=False)
    mskb8_d = nc.declare_dram_parameter("mskb8", [P, NC, CH], FP8, isOutput=False)
    mskabf_d = nc.declare_dram_parameter("mskabf", [P, P], BF16, isOutput=False)
    mskbbf_d = nc.declare_dram_parameter("mskbbf", [P, P], BF16, isOutput=False)
    ones8_d = nc.declare_dram_parameter("ones8", [P, 2, 4], FP8, isOutput=False)
    ones0_d = nc.declare_dram_parameter("ones0", [P, 4], F32R, isOutput=False)
    y_d = nc.declare_dram_parameter("y", [T, E], FP32, isOutput=True)

    with tile.TileContext(nc) as tc:
        with ExitStack() as ctx:
            # ---- persistent pools ----
            const_pool = ctx.enter_context(tc.tile_pool(name="const", bufs=1))
            bqk_pool = ctx.enter_context(tc.tile_pool(name="bqk", bufs=2 * NE))
            qk_pool = ctx.enter_context(tc.tile_pool(name="qk", bufs=2 * NA))
            qk0_pool = ctx.enter_context(tc.tile_pool(name="qk0", bufs=2 * NE))
            v_pool = ctx.enter_context(tc.tile_pool(name="v", bufs=NT // 2))
            psa = ctx.enter_context(tc.tile_pool(name="psa", bufs=3, space="PSUM"))

            ones8 = const_pool.tile([P, 2, 4], FP8, tag="ones8", name="ones8")
            nc.sync.dma_start(ones8[:], ones8_d[:])
            ones0 = const_pool.tile([P, 4], F32R, tag="ones0", name="ones0")
            nc.sync.dma_start(ones0[:], ones0_d[:])
            mska8 = const_pool.tile([P, P], FP8, tag="mska8", name="mska8")
            nc.sync.dma_start(mska8[:], mska8_d[:])
            mskb8 = const_pool.tile([P, NC, CH], FP8, tag="mskb8", name="mskb8")
            nc.sync.dma_start(mskb8[:], mskb8_d[:])
            mskabf = const_pool.tile([P, P], BF16, tag="mskabf", name="mskabf")
            nc.sync.dma_start(mskabf[:], mskabf_d[:])
            mskbbf = const_pool.tile([P, P], BF16, tag="mskbbf", name="mskbbf")
            nc.sync.dma_start(mskbbf[:], mskbbf_d[:])
            bvrep = const_pool.tile([P, E], FP32, tag="bvrep", name="bvrep")
            nc.sync.dma_start(bvrep[:], bvrep_d[:])

            # q pairs in qk8[0..NA), k pairs in qk8[NA..2NA)
            qk8 = [qk_pool.tile([P, 2, T], FP8, tag="qk8", name="qk8")
                   for _ in range(2 * NA)]
            qk0 = [qk0_pool.tile([P, P], BF16, tag="qk0", name="qk0")
                   for _ in range(2 * NE)]
            v8 = [v_pool.tile([P, 2, E], FP8, tag="v8", name="v8")
                  for _ in range(NT // 2)]
            v0 = const_pool.tile([P, E], F32R, tag="v0", name="v0")
            e0 = const_pool.tile([P, P], F32R, tag="e0", name="e0")

            # alternating copy-out engine dispatch (GPSIMD cannot read PSUM)
            def copyout_qk(i, dst, ps, bias_ap):
                if i % 2 == 0:
                    nc.scalar.activation(dst, ps, AF.Identity,
                                         bias=bias_ap, scale=1.0 / WS)
                else:
                    nc.vector.tensor_scalar(dst, ps, 1.0 / WS, bias_ap,
                                            op0=ALU.mult, op1=ALU.add)

            def copyout_v(i, dst, ps):
                if i % 2 == 0:
                    nc.scalar.activation(dst, ps, AF.Copy)
                else:
                    nc.vector.tensor_copy(dst, ps)

            # ---- phase 1: projections ----
            with ExitStack() as p1:
                xt8_pool = p1.enter_context(tc.tile_pool(name="xt8", bufs=NA))
                xtbf_pool = p1.enter_context(tc.tile_pool(name="xtbf", bufs=1))
                wqk8_pool = p1.enter_context(tc.tile_pool(name="wqk8p", bufs=4))
                wqkbf_pool = p1.enter_context(tc.tile_pool(name="wqkbfp", bufs=4))
                wv8_pool = p1.enter_context(tc.tile_pool(name="wv8p", bufs=NA))
                wvbf_pool = p1.enter_context(tc.tile_pool(name="wvbfp", bufs=1))

                # x first (critical path), split across sync + act queues
                xt8_sb = []
                for a2 in range(NA):
                    t_ = xt8_pool.tile([P, 2, T], FP8, tag="xt8", name="xt8")
                    nc.sync.dma_start(t_[:, 0, :], xt8_d[a2, :, 0, :])
                    nc.scalar.dma_start(t_[:, 1, :], xt8_d[a2, :, 1, :])
                    xt8_sb.append(t_)
                bqk_sb = []
                for ft in range(2 * NE):
                    t_ = bqk_pool.tile([P, 1], FP32, tag="bqk", name="bqk")
                    nc.sync.dma_start(t_[:], bqk_d[ft])
                    bqk_sb.append(t_)
                xtbf_sb = xtbf_pool.tile([P, NE, P], BF16, tag="xtbf", name="xtbf")
                nc.sync.dma_start(xtbf_sb[:], xtbf_d[:])

                # q^T, k^T fp8 pair-tiles, f-tile by f-tile
                wv8_sb = None
                for ft in range(2 * NE):
                    wt = wqk8_pool.tile([P, NA, 2, P], FP8, tag="wqk8", name="wqk8")
                    nc.scalar.dma_start(wt[:], wqk8_d[ft])
                    if ft == 2:
                        wv8_sb = []
                        for a2 in range(NA):
                            t_ = wv8_pool.tile([P, 2, E], FP8, tag="wv8", name="wv8")
                            nc.scalar.dma_start(t_[:], wv8_d[a2])
                            wv8_sb.append(t_)
                    for tch in range(NC):
                        ps = psa.tile([P, CH], FP32, tag="ps", name="ps")
                        for a2 in range(NA):
                            nc.tensor.matmul(
                                ps[:],
                                lhsT=wt[:, a2, :, :],
                                rhs=xt8_sb[a2][:, :, tch * CH:(tch + 1) * CH],
                                start=(a2 == 0),
                                stop=(a2 == NA - 1),
                                perf_mode=DR,
                            )
                        g, i = ft // 2, ft % 2
                        copyout_qk(ft * NC + tch,
                                   qk8[g][:, i, tch * CH:(tch + 1) * CH],
                                   ps[:], bqk_sb[ft][:])

                # high-precision q0,k0 (bf16, t<128 output columns)
                for ft in range(2 * NE):
                    wbf = wqkbf_pool.tile([P, NE, P], BF16, tag="wqkbf", name="wqkbf")
                    nc.gpsimd.dma_start(wbf[:], wqkbf_d[ft])
                    ps = psa.tile([P, CH], FP32, tag="ps", name="ps")
                    for e in range(NE):
                        nc.tensor.matmul(
                            ps[:, 0:P],
                            lhsT=wbf[:, e, :],
                            rhs=xtbf_sb[:, e, :],
                            start=(e == 0),
                            stop=(e == NE - 1),
                        )
                    nc.scalar.activation(qk0[ft][:], ps[:, 0:P], AF.Identity,
                                         bias=bqk_sb[ft][:], scale=1.0)

                # high-precision scores+exp for the t<128 block
                ps = psa.tile([P, CH], FP32, tag="ps", name="ps")
                for e in range(NE):
                    nc.tensor.matmul(ps[:, 0:P], lhsT=qk0[NE + e][:],
                                     rhs=qk0[e][:],
                                     start=(e == 0), stop=False)
                nc.tensor.matmul(ps[:, 0:P], lhsT=mskabf[:], rhs=mskbbf[:],
                                 start=False, stop=True)
                nc.scalar.activation(e0[:], ps[:, 0:P], AF.Exp, scale=SCALE)

                # v in [t, e] layout: fp8 (x16) for all tiles + fp32r tile 0
                wvbf_sb = wvbf_pool.tile([P, NE, E], BF16, tag="wvbf", name="wvbf")
                for e in range(NE):
                    nc.gpsimd.dma_start(wvbf_sb[:, e, :], wvbf_d[:, e, :])
                for tt in range(NT):
                    for ec in range(2):
                        ps = psa.tile([P, CH], FP32, tag="ps", name="ps")
                        for a2 in range(NA):
                            nc.tensor.matmul(
                                ps[:],
                                lhsT=xt8_sb[a2][:, :, tt * P:(tt + 1) * P],
                                rhs=wv8_sb[a2][:, :, ec * CH:(ec + 1) * CH],
                                start=(a2 == 0),
                                stop=(a2 == NA - 1),
                                perf_mode=DR,
                            )
                        copyout_v(tt * 2 + ec,
                                  v8[tt // 2][:, tt % 2, ec * CH:(ec + 1) * CH],
                                  ps[:])
                for ec in range(2):
                    ps = psa.tile([P, CH], FP32, tag="ps", name="ps")
                    for e in range(NE):
                        nc.tensor.matmul(
                            ps[:],
                            lhsT=xtbf_sb[:, e, :],
                            rhs=wvbf_sb[:, e, ec * CH:(ec + 1) * CH],
                            start=(e == 0),
                            stop=(e == NE - 1),
                        )
                    nc.scalar.activation(v0[:, ec * CH:(ec + 1) * CH], ps[:],
                                         AF.Copy)

            # ---- phases 2+3: scores+softmax+PV ----
            with ExitStack() as p2:
                exps_pool = p2.enter_context(tc.tile_pool(name="exps", bufs=10))
                y_pool = p2.enter_context(tc.tile_pool(name="yst", bufs=3))
                zr_pool = p2.enter_context(tc.tile_pool(name="zr", bufs=4))
                psy = p2.enter_context(tc.tile_pool(name="psy", bufs=3, space="PSUM"))
                psz = p2.enter_context(tc.tile_pool(name="psz", bufs=2, space="PSUM"))

                # high-precision y rows 0..127
                ps_z = psz.tile([P, 4 * NC], FP32, tag="z", name="z")
                nc.tensor.matmul(ps_z[:, 0:4], lhsT=e0[:], rhs=ones0[:],
                                 start=True, stop=True)
                zr0 = zr_pool.tile([P, 1], FP32, tag="zr", name="zr")
                nc.vector.reciprocal(zr0[:], ps_z[:, 0:1])
                y_t0 = y_pool.tile([P, E], FP32, tag="y", name="y")
                for ec in range(2):
                    ps_y = psy.tile([P, CH], FP32, tag="y", name="psy")
                    nc.tensor.matmul(ps_y[:], lhsT=e0[:],
                                     rhs=v0[:, ec * CH:(ec + 1) * CH],
                                     start=True, stop=True)
                    nc.vector.scalar_tensor_tensor(
                        y_t0[:, ec * CH:(ec + 1) * CH], ps_y[:], zr0[:],
                        bvrep[:, ec * CH:(ec + 1) * CH],
                        op0=ALU.mult, op1=ALU.add)
                nc.sync.dma_start(y_d[0:P, :], y_t0[:])

                for c in range(NC):
                    n_tk = (c + 1) * (CH // P)  # tk tiles at/below diagonal
                    es = []
                    for tk in range(n_tk):
                        if tk % 2 == 0:
                            es.append(exps_pool.tile([P, 2, CH], FP8,
                                                     tag="es", name="es"))
                        ps = psa.tile([P, CH], FP32, tag="ps", name="ps")
                        d = tk - c * (CH // P)
                        for a2 in range(NA):
                            nc.tensor.matmul(
                                ps[:],
                                lhsT=qk8[NA + a2][:, :, tk * P:(tk + 1) * P],
                                rhs=qk8[a2][:, :, c * CH:(c + 1) * CH],
                                start=(a2 == 0),
                                stop=(a2 == NA - 1) and d < 0,
                                perf_mode=DR,
                            )
                        if d >= 0:  # diagonal-crossing tile: additive mask matmul
                            nc.tensor.matmul(ps[:], lhsT=mska8[:],
                                             rhs=mskb8[:, d, :],
                                             start=False, stop=True)
                        nc.scalar.activation(es[tk // 2][:, tk % 2, :], ps[:],
                                             AF.Exp, scale=SCALE)

                    ps_z = psz.tile([P, 4 * NC], FP32, tag="z", name="z")
                    for j in range(NC):
                        tq = c * (CH // P) + j
                        if tq == 0:
                            continue  # high-precision path above
                        nu = tq // 2 + 1  # pair-tiles; odd-index tail is 0 in fp8
                        for u in range(nu):
                            nc.tensor.matmul(
                                ps_z[:, 4 * j:4 * j + 4],
                                lhsT=es[u][:, :, j * P:(j + 1) * P],
                                rhs=ones8[:],
                                start=(u == 0),
                                stop=(u == nu - 1),
                                perf_mode=DR,
                            )
                        zr = zr_pool.tile([P, 1], FP32, tag="zr", name="zr")
                        nc.vector.reciprocal(zr[:], ps_z[:, 4 * j:4 * j + 1])
                        y_t = y_pool.tile([P, E], FP32, tag="y", name="y")
                        for ec in range(2):
                            ps_y = psy.tile([P, CH], FP32, tag="y", name="psy")
                            for u in range(nu):
                                nc.tensor.matmul(
                                    ps_y[:],
                                    lhsT=es[u][:, :, j * P:(j + 1) * P],
                                    rhs=v8[u][:, :, ec * CH:(ec + 1) * CH],
                                    start=(u == 0),
                                    stop=(u == nu - 1),
                                    perf_mode=DR,
                                )
                            nc.vector.scalar_tensor_tensor(
                                y_t[:, ec * CH:(ec + 1) * CH], ps_y[:], zr[:],
                                bvrep[:, ec * CH:(ec + 1) * CH],
                                op0=ALU.mult, op1=ALU.add)
                        nc.sync.dma_start(y_d[tq * P:(tq + 1) * P, :], y_t[:])
    nc.finalize()
    return nc


_NC_CACHE = {}


def _get_nc():
    if "nc" not in _NC_CACHE:
        _NC_CACHE["nc"] = _build_nc()
    return _NC_CACHE["nc"]


def _prep_inputs(x, W, b):
    # xt8[a2, p, i, t] = x[t, (2*a2+i)*128+p]
    xt = np.ascontiguousarray(
        x.reshape(B, T, NA, 2, P).transpose(0, 2, 4, 3, 1))       # [B,NA,P,2,T]
    xt8 = xt.astype(E4NP)
    # xtbf[p, a, t] = x[t<128, a*128+p]
    xtbf = np.ascontiguousarray(
        x[:, :P].reshape(B, P, NE, P).transpose(0, 3, 2, 1)).astype(BFNP)
    # wqk8[ft, p, a2, i, f] = 16*W[ft*128+f, (2*a2+i)*128+p]
    wqk8 = np.ascontiguousarray(
        (W[:2 * E] * WS).reshape(2 * NE, P, NA, 2, P).transpose(0, 4, 2, 3, 1)
    ).astype(E4NP)
    # wqkbf[ft, p, a, f] = W[ft*128+f, a*128+p]
    wqkbf = np.ascontiguousarray(
        W[:2 * E].reshape(2 * NE, P, NE, P).transpose(0, 3, 2, 1)).astype(BFNP)
    # wv8[a2, p, i, eo] = 16*W[2E+eo, (2*a2+i)*128+p]
    wv8 = np.ascontiguousarray(
        (W[2 * E:] * WS).reshape(E, NA, 2, P).transpose(1, 3, 2, 0)).astype(E4NP)
    # wvbf[p, a, eo] = 16*W[2E+eo, a*128+p]  (x16 so v0 matches v8 scaling)
    wvbf = np.ascontiguousarray(
        (W[2 * E:] * WS).reshape(E, NE, P).transpose(2, 1, 0)).astype(BFNP)
    bqk = b[:2 * E].astype(np.float32).reshape(2 * NE, P, 1).copy()
    bvrep = np.broadcast_to(b[2 * E:].astype(np.float32), (P, E)).copy()
    mska8 = (MA * np.eye(P, dtype=np.float32)).astype(E4NP)
    ii = np.arange(P)[:, None]
    jj = np.arange(CH)[None, :]
    # mskb8[r, d, j] = MB if j < d*128 + r else 0
    mskb8 = np.ascontiguousarray(np.stack(
        [np.where(jj < d * P + ii, MB, 0.0) for d in range(NC)]
    ).transpose(1, 0, 2)).astype(np.float32).astype(E4NP)
    mskabf = (MA * np.eye(P, dtype=np.float32)).astype(BFNP)
    mskbbf = np.where(jj[:, :P] < ii, MB, 0.0).astype(np.float32).astype(BFNP)
    ones8 = np.full((P, 2, 4), WS, np.float32).astype(E4NP)
    ones0 = np.full((P, 4), WS, np.float32)
    shared = {"wqk8": wqk8, "wqkbf": wqkbf, "wv8": wv8, "wvbf": wvbf,
              "bqk": bqk, "bvrep": bvrep, "mska8": mska8, "mskb8": mskb8,
              "mskabf": mskabf, "mskbbf": mskbbf, "ones8": ones8,
              "ones0": ones0}
    return [{"xt8": np.ascontiguousarray(xt8[i]),
             "xtbf": np.ascontiguousarray(xtbf[i]), **shared}
            for i in range(B)]


def run(x, W, b, **spmd_kwargs):
    nc = _get_nc()
    in_maps = _prep_inputs(np.asarray(x, np.float32), np.asarray(W, np.float32),
                           np.asarray(b, np.float32))
    res = run_bass_kernel_spmd(nc, in_maps, list(range(B)), **spmd_kwargs)
    y = np.stack([res.results[i]["y"] for i in range(B)]).astype(np.float32)
    return y, res


def kernel(x, W, b):
    y, _ = run(x, W, b)
    return y


# revision 10
# speedup vs baseline: 2.4685x; 1.0204x over previous
"""Single-head causal attention block (QKV projection + attention) on 8 TRN2 cores.

Reference computation (per batch element b, batch-sharded 1 core each):
    qkv = x[b] @ W.T + b          # [T, 3E]
    q, k, v = split(qkv)          # each [T, E]
    s = (q @ k.T) / sqrt(E), causal-masked
    y = softmax(s) @ v            # [T, E]

Shapes: B=8, T=2048, E=1024.

Speed strategy: nearly all matmul FLOPs run as fp8(e4m3) DoubleRow matmuls
(0.5 cycles/row = 2x bf16 rate).  DoubleRow contracts 256 elements per
instruction: both operands carry a pair dim in a 3D AP [128, 2, N].

Accuracy strategy (rel-err budget 2e-2, achieved ~5.8e-3 on hw):
  - W scaled x16 before e4m3 quantization (avoids subnormals); /16 folded
    into the copy-out (q,k) or into the ones/Z scaling (v path: v stored x16,
    Z computed with ones=16, so y = ps_y * (1/(16Z)) is exact).
  - q,k stored unscaled e4m3 (std ~0.58); the 1/sqrt(E) score scale is
    applied inside the Exp activation (out = exp(in*scale)).
  - causal mask applied as an extra accumulating PE matmul: (-16*I).T @
    (96*tril-indicator) = -1536 on masked entries -> exp(scale*s - 48) -> 0
    exactly in fp8.  The indicator is zero beyond column (d+1)*128, so the
    mask matmul only covers that prefix.
  - softmax row sums Z from a ones(=16)-column DoubleRow matmul; masked
    entries quantize to exactly 0 so above-diagonal pair-tiles contribute 0,
    letting every PV/Z accumulation use full pairs.
  - rows t<128 (few softmax terms -> no error averaging) are recomputed on a
    high-precision bf16/fp32r path: bf16 q0,k0,v0 projections, fp32 exp,
    fp32r PV.  This bounds the worst-case row error; rows >=128 average fp8
    noise over >=129 keys.

Schedule notes: PSUM copy-outs are split into half-tiles drained by ACT and
DVE in parallel (each half ~360-390ns vs the PE's 427ns per psum group, so
phase 1 stays PE-bound).  PV/Z for chunk c-1 are emitted after scores/exp of
chunk c so the ACT exp lag never stalls the PE.  Constants stream on the
gpsimd SWDGE queue, keeping the HWDGE path clear for x/W at startup.
"""

import numpy as np
import ml_dtypes
from contextlib import ExitStack

import concourse.bass as bass
import concourse.bacc as bacc
import concourse.mybir as mybir
import concourse.tile as tile
from concourse.bass_utils import run_bass_kernel_spmd

FP32 = mybir.dt.float32
F32R = mybir.dt.float32r
BF16 = mybir.dt.bfloat16
FP8 = mybir.dt.float8e4
AF = mybir.ActivationFunctionType
ALU = mybir.AluOpType
DR = mybir.MatmulPerfMode.DoubleRow

B, T, E = 8, 2048, 1024
P = 128
NE = E // P            # 8 e-tiles (contraction)
NA = NE // 2           # 4 e-pair tiles for DoubleRow
NT = T // P            # 16 t-tiles
NC = 4                 # tq chunks of 512
CH = T // NC           # 512
HC = CH // 2           # 256 (half-tile copyout split)
SCALE = 1.0 / np.sqrt(E)
WS = 16.0              # weight pre-scale before fp8 quantization
MA = -16.0             # mask matmul stationary value ...
MB = 96.0              # ... times moving value = -1536 on masked entries

E4NP = ml_dtypes.float8_e4m3fn
BFNP = ml_dtypes.bfloat16


def _build_nc():
    nc = bacc.Bacc()

    xt8_d = nc.declare_dram_parameter("xt8", [NA, P, 2, T], FP8, isOutput=False)
    xtbf_d = nc.declare_dram_parameter("xtbf", [P, NE, P], BF16, isOutput=False)
    wqk8_d = nc.declare_dram_parameter("wqk8", [2 * NE, P, NA, 2, P], FP8, isOutput=False)
    wqkbf_d = nc.declare_dram_parameter("wqkbf", [2 * NE, P, NE, P], BF16, isOutput=False)
    wv8_d = nc.declare_dram_parameter("wv8", [NA, P, 2, E], FP8, isOutput=False)
    wvbf_d = nc.declare_dram_parameter("wvbf", [P, NE, E], BF16, isOutput=False)
    bqk_d = nc.declare_dram_parameter("bqk", [P, 2 * NE], FP32, isOutput=False)
    bvrep_d = nc.declare_dram_parameter("bvrep", [P, E], FP32, isOutput=False)
    mska8_d = nc.declare_dram_parameter("mska8", [P, P], FP8, isOutput=False)
    mskb8_d = nc.declare_dram_parameter("mskb8", [P, NC, CH], FP8, isOutput=False)
    mskabf_d = nc.declare_dram_parameter("mskabf", [P, P], BF16, isOutput=False)
    mskbbf_d = nc.declare_dram_parameter("mskbbf", [P, P], BF16, isOutput=False)
    ones8_d = nc.declare_dram_parameter("ones8", [P, 2, 4], FP8, isOutput=False)
    ones0_d = nc.declare_dram_parameter("ones0", [P, 4], F32R, isOutput=False)
    y_d = nc.declare_dram_parameter("y", [T, E], FP32, isOutput=True)

    with tile.TileContext(nc) as tc:
        with ExitStack() as ctx:
            # ---- persistent pools ----
            const_pool = ctx.enter_context(tc.tile_pool(name="const", bufs=1))
            qk_pool = ctx.enter_context(tc.tile_pool(name="qk", bufs=2 * NA))
            qk0_pool = ctx.enter_context(tc.tile_pool(name="qk0", bufs=2 * NE))
            v_pool = ctx.enter_context(tc.tile_pool(name="v", bufs=NT // 2))
            psa = ctx.enter_context(tc.tile_pool(name="psa", bufs=3, space="PSUM"))

            # q pairs in qk8[0..NA), k pairs in qk8[NA..2NA)
            qk8 = [qk_pool.tile([P, 2, T], FP8, tag="qk8", name="qk8")
                   for _ in range(2 * NA)]
            qk0 = [qk0_pool.tile([P, P], BF16, tag="qk0", name="qk0")
                   for _ in range(2 * NE)]
            v8 = [v_pool.tile([P, 2, E], FP8, tag="v8", name="v8")
                  for _ in range(NT // 2)]
            v0 = const_pool.tile([P, E], F32R, tag="v0", name="v0")
            e0 = const_pool.tile([P, P], F32R, tag="e0", name="e0")

            # split copy-out: ACT takes the low half, DVE the high half
            def copyout_qk(dst_lo, dst_hi, ps, bias_ap):
                nc.scalar.activation(dst_lo, ps[:, 0:HC], AF.Identity,
                                     bias=bias_ap, scale=1.0 / WS)
                nc.vector.tensor_scalar(dst_hi, ps[:, HC:CH], 1.0 / WS, bias_ap,
                                        op0=ALU.mult, op1=ALU.add)

            def copyout_v(dst_lo, dst_hi, ps):
                nc.scalar.activation(dst_lo, ps[:, 0:HC], AF.Copy)
                nc.vector.tensor_copy(dst_hi, ps[:, HC:CH])

            # ---- phase 1: projections ----
            with ExitStack() as p1:
                xt8_pool = p1.enter_context(tc.tile_pool(name="xt8", bufs=NA))
                xtbf_pool = p1.enter_context(tc.tile_pool(name="xtbf", bufs=1))
                wqk8_pool = p1.enter_context(tc.tile_pool(name="wqk8p", bufs=6))
                wqkbf_pool = p1.enter_context(tc.tile_pool(name="wqkbfp", bufs=2 * NE))
                wv8_pool = p1.enter_context(tc.tile_pool(name="wv8p", bufs=NA))
                wvbf_pool = p1.enter_context(tc.tile_pool(name="wvbfp", bufs=1))

                # x + first weights first (critical path), sync + act queues
                xt8_sb = []
                for a2 in range(NA):
                    t_ = xt8_pool.tile([P, 2, T], FP8, tag="xt8", name="xt8")
                    q = nc.sync if a2 % 2 == 0 else nc.scalar
                    q.dma_start(t_[:], xt8_d[a2])
                    xt8_sb.append(t_)
                bqk_sb = const_pool.tile([P, 2 * NE], FP32, tag="bqk", name="bqk")
                nc.sync.dma_start(bqk_sb[:], bqk_d[:])
                xtbf_sb = xtbf_pool.tile([P, NE, P], BF16, tag="xtbf", name="xtbf")
                nc.sync.dma_start(xtbf_sb[:], xtbf_d[:])

                # constants stream on the gpsimd SWDGE queue (HWDGE stays clear)
                ones8 = const_pool.tile([P, 2, 4], FP8, tag="ones8", name="ones8")
                nc.gpsimd.dma_start(ones8[:], ones8_d[:])
                ones0 = const_pool.tile([P, 4], F32R, tag="ones0", name="ones0")
                nc.gpsimd.dma_start(ones0[:], ones0_d[:])
                mska8 = const_pool.tile([P, P], FP8, tag="mska8", name="mska8")
                nc.gpsimd.dma_start(mska8[:], mska8_d[:])
                mskb8 = const_pool.tile([P, NC, CH], FP8, tag="mskb8", name="mskb8")
                nc.gpsimd.dma_start(mskb8[:], mskb8_d[:])
                mskabf = const_pool.tile([P, P], BF16, tag="mskabf", name="mskabf")
                nc.gpsimd.dma_start(mskabf[:], mskabf_d[:])
                mskbbf = const_pool.tile([P, P], BF16, tag="mskbbf", name="mskbbf")
                nc.gpsimd.dma_start(mskbbf[:], mskbbf_d[:])
                bvrep = const_pool.tile([P, E], FP32, tag="bvrep", name="bvrep")
                nc.gpsimd.dma_start(bvrep[:], bvrep_d[:])

                # q^T, k^T fp8 pair-tiles, f-tile by f-tile
                wv8_sb = None
                for ft in range(2 * NE):
                    wt = wqk8_pool.tile([P, NA, 2, P], FP8, tag="wqk8", name="wqk8")
                    nc.scalar.dma_start(wt[:], wqk8_d[ft])
                    if ft == 2:
                        wv8_sb = []
                        for a2 in range(NA):
                            t_ = wv8_pool.tile([P, 2, E], FP8, tag="wv8", name="wv8")
                            nc.scalar.dma_start(t_[:], wv8_d[a2])
                            wv8_sb.append(t_)
                    for tch in range(NC):
                        ps = psa.tile([P, CH], FP32, tag="ps", name="ps")
                        for a2 in range(NA):
                            nc.tensor.matmul(
                                ps[:],
                                lhsT=wt[:, a2, :, :],
                                rhs=xt8_sb[a2][:, :, tch * CH:(tch + 1) * CH],
                                start=(a2 == 0),
                                stop=(a2 == NA - 1),
                                perf_mode=DR,
                            )
                        g, i = ft // 2, ft % 2
                        c0 = tch * CH
                        copyout_qk(qk8[g][:, i, c0:c0 + HC],
                                   qk8[g][:, i, c0 + HC:c0 + CH],
                                   ps, bqk_sb[:, ft:ft + 1])

                # high-precision q0,k0 (bf16, t<128 output columns);
                # weight tiles prefetched on the sync queue
                wbf_sb = []
                for ft in range(2 * NE):
                    wbf = wqkbf_pool.tile([P, NE, P], BF16, tag="wqkbf", name="wqkbf")
                    nc.sync.dma_start(wbf[:], wqkbf_d[ft])
                    wbf_sb.append(wbf)
                for ft in range(2 * NE):
                    ps = psa.tile([P, CH], FP32, tag="ps", name="ps")
                    for e in range(NE):
                        nc.tensor.matmul(
                            ps[:, 0:P],
                            lhsT=wbf_sb[ft][:, e, :],
                            rhs=xtbf_sb[:, e, :],
                            start=(e == 0),
                            stop=(e == NE - 1),
                        )
                    nc.scalar.activation(qk0[ft][:], ps[:, 0:P], AF.Identity,
                                         bias=bqk_sb[:, ft:ft + 1], scale=1.0)

                # high-precision scores+exp for the t<128 block
                ps = psa.tile([P, CH], FP32, tag="ps", name="ps")
                for e in range(NE):
                    nc.tensor.matmul(ps[:, 0:P], lhsT=qk0[NE + e][:],
                                     rhs=qk0[e][:],
                                     start=(e == 0), stop=False)
                nc.tensor.matmul(ps[:, 0:P], lhsT=mskabf[:], rhs=mskbbf[:],
                                 start=False, stop=True)
                nc.scalar.activation(e0[:], ps[:, 0:P], AF.Exp, scale=SCALE)

                # v in [t, e] layout: fp8 (x16) for all tiles + fp32r tile 0
                wvbf_sb = wvbf_pool.tile([P, NE, E], BF16, tag="wvbf", name="wvbf")
                for e in range(NE):
                    nc.gpsimd.dma_start(wvbf_sb[:, e, :], wvbf_d[:, e, :])
                for tt in range(NT):
                    for ec in range(2):
                        ps = psa.tile([P, CH], FP32, tag="ps", name="ps")
                        for a2 in range(NA):
                            nc.tensor.matmul(
                                ps[:],
                                lhsT=xt8_sb[a2][:, :, tt * P:(tt + 1) * P],
                                rhs=wv8_sb[a2][:, :, ec * CH:(ec + 1) * CH],
                                start=(a2 == 0),
                                stop=(a2 == NA - 1),
                                perf_mode=DR,
                            )
                        c0 = ec * CH
                        copyout_v(v8[tt // 2][:, tt % 2, c0:c0 + HC],
                                  v8[tt // 2][:, tt % 2, c0 + HC:c0 + CH],
                                  ps)
                for ec in range(2):
                    ps = psa.tile([P, CH], FP32, tag="ps", name="ps")
                    for e in range(NE):
                        nc.tensor.matmul(
                            ps[:],
                            lhsT=xtbf_sb[:, e, :],
                            rhs=wvbf_sb[:, e, ec * CH:(ec + 1) * CH],
                            start=(e == 0),
                            stop=(e == NE - 1),
                        )
                    nc.scalar.activation(v0[:, ec * CH:(ec + 1) * CH], ps[:],
                                         AF.Copy)

            # ---- phases 2+3: scores+softmax+PV ----
            with ExitStack() as p2:
                exps_pool = p2.enter_context(tc.tile_pool(name="exps", bufs=16))
                y_pool = p2.enter_context(tc.tile_pool(name="yst", bufs=3))
                zr_pool = p2.enter_context(tc.tile_pool(name="zr", bufs=4))
                psy = p2.enter_context(tc.tile_pool(name="psy", bufs=3, space="PSUM"))
                psz = p2.enter_context(tc.tile_pool(name="psz", bufs=2, space="PSUM"))

                # high-precision y rows 0..127
                ps_z = psz.tile([P, 4 * NC], FP32, tag="z", name="z")
                nc.tensor.matmul(ps_z[:, 0:4], lhsT=e0[:], rhs=ones0[:],
                                 start=True, stop=True)
                zr0 = zr_pool.tile([P, 1], FP32, tag="zr", name="zr")
                nc.vector.reciprocal(zr0[:], ps_z[:, 0:1])
                y_t0 = y_pool.tile([P, E], FP32, tag="y", name="y")
                for ec in range(2):
                    ps_y = psy.tile([P, CH], FP32, tag="y", name="psy")
                    nc.tensor.matmul(ps_y[:], lhsT=e0[:],
                                     rhs=v0[:, ec * CH:(ec + 1) * CH],
                                     start=True, stop=True)
                    nc.vector.scalar_tensor_tensor(
                        y_t0[:, ec * CH:(ec + 1) * CH], ps_y[:], zr0[:],
                        bvrep[:, ec * CH:(ec + 1) * CH],
                        op0=ALU.mult, op1=ALU.add)
                    nc.sync.dma_start(y_d[0:P, ec * CH:(ec + 1) * CH],
                                      y_t0[:, ec * CH:(ec + 1) * CH])

                def scores_chunk(c):
                    n_tk = (c + 1) * (CH // P)
                    es = []
                    for tk in range(n_tk):
                        if tk % 2 == 0:
                            es.append(exps_pool.tile([P, 2, CH], FP8,
                                                     tag="es", name="es"))
                        ps = psa.tile([P, CH], FP32, tag="ps", name="ps")
                        d = tk - c * (CH // P)
                        for a2 in range(NA):
                            # mask matmul slots in before the last accumulate
                            if d >= 0 and a2 == NA - 1:
                                nc.tensor.matmul(
                                    ps[:, 0:(d + 1) * P], lhsT=mska8[:],
                                    rhs=mskb8[:, d, 0:(d + 1) * P],
                                    start=False, stop=False)
                            nc.tensor.matmul(
                                ps[:],
                                lhsT=qk8[NA + a2][:, :, tk * P:(tk + 1) * P],
                                rhs=qk8[a2][:, :, c * CH:(c + 1) * CH],
                                start=(a2 == 0),
                                stop=(a2 == NA - 1),
                                perf_mode=DR,
                            )
                        nc.scalar.activation(es[tk // 2][:, tk % 2, :], ps[:],
                                             AF.Exp, scale=SCALE)
                    return es

                def pv_chunk(c, es, js):
                    ps_z = psz.tile([P, 4 * NC], FP32, tag="z", name="z")
                    for j in js:
                        tq = c * (CH // P) + j
                        if tq == 0:
                            continue  # high-precision path above
                        nu = tq // 2 + 1  # pair-tiles; odd tail is 0 in fp8
                        for u in range(nu):
                            nc.tensor.matmul(
                                ps_z[:, 4 * j:4 * j + 4],
                                lhsT=es[u][:, :, j * P:(j + 1) * P],
                                rhs=ones8[:],
                                start=(u == 0),
                                stop=(u == nu - 1),
                                perf_mode=DR,
                            )
                        zr = zr_pool.tile([P, 1], FP32, tag="zr", name="zr")
                        nc.vector.reciprocal(zr[:], ps_z[:, 4 * j:4 * j + 1])
                        y_t = y_pool.tile([P, E], FP32, tag="y", name="y")
                        for ec in range(2):
                            ps_y = psy.tile([P, CH], FP32, tag="y", name="psy")
                            for u in range(nu):
                                nc.tensor.matmul(
                                    ps_y[:],
                                    lhsT=es[u][:, :, j * P:(j + 1) * P],
                                    rhs=v8[u][:, :, ec * CH:(ec + 1) * CH],
                                    start=(u == 0),
                                    stop=(u == nu - 1),
                                    perf_mode=DR,
                                )
                            nc.vector.scalar_tensor_tensor(
                                y_t[:, ec * CH:(ec + 1) * CH], ps_y[:], zr[:],
                                bvrep[:, ec * CH:(ec + 1) * CH],
                                op0=ALU.mult, op1=ALU.add)
                            nc.sync.dma_start(
                                y_d[tq * P:(tq + 1) * P, ec * CH:(ec + 1) * CH],
                                y_t[:, ec * CH:(ec + 1) * CH])

                # software pipeline: PV of chunk c-1 runs under scores of c
                es_prev = None
                for c in range(NC):
                    es_cur = scores_chunk(c)
                    if es_prev is not None:
                        pv_chunk(c - 1, es_prev, range(NC))
                    es_prev = es_cur
                # last chunk: cheapest tile last to shorten the tail
                pv_chunk(NC - 1, es_prev, [1, 2, 3, 0])
    nc.finalize()
    return nc


_NC_CACHE = {}


def _get_nc():
    if "nc" not in _NC_CACHE:
        _NC_CACHE["nc"] = _build_nc()
    return _NC_CACHE["nc"]


def _prep_inputs(x, W, b):
    # xt8[a2, p, i, t] = x[t, (2*a2+i)*128+p]
    xt = np.ascontiguousarray(
        x.reshape(B, T, NA, 2, P).transpose(0, 2, 4, 3, 1))       # [B,NA,P,2,T]
    xt8 = xt.astype(E4NP)
    # xtbf[p, a, t] = x[t<128, a*128+p]
    xtbf = np.ascontiguousarray(
        x[:, :P].reshape(B, P, NE, P).transpose(0, 3, 2, 1)).astype(BFNP)
    # wqk8[ft, p, a2, i, f] = 16*W[ft*128+f, (2*a2+i)*128+p]
    wqk8 = np.ascontiguousarray(
        (W[:2 * E] * WS).reshape(2 * NE, P, NA, 2, P).transpose(0, 4, 2, 3, 1)
    ).astype(E4NP)
    # wqkbf[ft, p, a, f] = W[ft*128+f, a*128+p]
    wqkbf = np.ascontiguousarray(
        W[:2 * E].reshape(2 * NE, P, NE, P).transpose(0, 3, 2, 1)).astype(BFNP)
    # wv8[a2, p, i, eo] = 16*W[2E+eo, (2*a2+i)*128+p]
    wv8 = np.ascontiguousarray(
        (W[2 * E:] * WS).reshape(E, NA, 2, P).transpose(1, 3, 2, 0)).astype(E4NP)
    # wvbf[p, a, eo] = 16*W[2E+eo, a*128+p]  (x16 so v0 matches v8 scaling)
    wvbf = np.ascontiguousarray(
        (W[2 * E:] * WS).reshape(E, NE, P).transpose(2, 1, 0)).astype(BFNP)
    # bqk[p, ft] = b[ft*128+p]
    bqk = np.ascontiguousarray(
        b[:2 * E].astype(np.float32).reshape(2 * NE, P).T).copy()
    bvrep = np.broadcast_to(b[2 * E:].astype(np.float32), (P, E)).copy()
    mska8 = (MA * np.eye(P, dtype=np.float32)).astype(E4NP)
    ii = np.arange(P)[:, None]
    jj = np.arange(CH)[None, :]
    # mskb8[r, d, j] = MB if j < d*128 + r else 0
    mskb8 = np.ascontiguousarray(np.stack(
        [np.where(jj < d * P + ii, MB, 0.0) for d in range(NC)]
    ).transpose(1, 0, 2)).astype(np.float32).astype(E4NP)
    mskabf = (MA * np.eye(P, dtype=np.float32)).astype(BFNP)
    mskbbf = np.where(jj[:, :P] < ii, MB, 0.0).astype(np.float32).astype(BFNP)
    ones8 = np.full((P, 2, 4), WS, np.float32).astype(E4NP)
    ones0 = np.full((P, 4), WS, np.float32)
    shared = {"wqk8": wqk8, "wqkbf": wqkbf, "wv8": wv8, "wvbf": wvbf,
              "bqk": bqk, "bvrep": bvrep, "mska8": mska8, "mskb8": mskb8,
              "mskabf": mskabf, "mskbbf": mskbbf, "ones8": ones8,
              "ones0": ones0}
    return [{"xt8": np.ascontiguousarray(xt8[i]),
             "xtbf": np.ascontiguousarray(xtbf[i]), **shared}
            for i in range(B)]


def run(x, W, b, **spmd_kwargs):
    nc = _get_nc()
    in_maps = _prep_inputs(np.asarray(x, np.float32), np.asarray(W, np.float32),
                           np.asarray(b, np.float32))
    res = run_bass_kernel_spmd(nc, in_maps, list(range(B)), **spmd_kwargs)
    y = np.stack([res.results[i]["y"] for i in range(B)]).astype(np.float32)
    return y, res


def kernel(x, W, b):
    y, _ = run(x, W, b)
    return y


# revision 14
# speedup vs baseline: 2.7482x; 1.1133x over previous
"""Single-head causal attention block (QKV projection + attention) on 8 TRN2 cores.

Reference computation (per batch element b, batch-sharded 1 core each):
    qkv = x[b] @ W.T + b          # [T, 3E]
    q, k, v = split(qkv)          # each [T, E]
    s = (q @ k.T) / sqrt(E), causal-masked
    y = softmax(s) @ v            # [T, E]

Shapes: B=8, T=2048, E=1024.

Speed strategy: nearly all matmul FLOPs run as fp8(e4m3) DoubleRow matmuls
(0.5 cycles/row = 2x bf16 rate).  DoubleRow contracts 256 elements per
instruction: both operands carry a pair dim in a 3D AP [128, 2, N].

Accuracy strategy (rel-err budget 2e-2, achieved ~5.8e-3 on hw):
  - W scaled x16 before e4m3 quantization (avoids subnormals); /16 folded
    into the copy-out (q,k) or into the ones/Z scaling (v path: v stored x16,
    Z computed with ones=16, so y = ps_y * (1/(16Z)) is exact).
  - q,k stored unscaled e4m3 (std ~0.58); the 1/sqrt(E) score scale is
    applied inside the Exp activation (out = exp(in*scale)).
  - causal mask applied as an extra accumulating PE matmul: (-16*I).T @
    (96*tril-indicator) = -1536 on masked entries -> exp(scale*s - 48) -> 0
    exactly in fp8.  The indicator is zero beyond column (d+1)*128, so the
    mask matmul only covers that prefix.
  - softmax row sums Z from a ones(=16)-column DoubleRow matmul; masked
    entries quantize to exactly 0 so above-diagonal pair-tiles contribute 0,
    letting every PV/Z accumulation use full pairs.
  - rows t<128 (few softmax terms -> no error averaging) are recomputed on a
    high-precision bf16/fp32r path: bf16 q0,k0,v0 projections, fp32 exp,
    fp32r PV.  This bounds the worst-case row error; rows >=128 average fp8
    noise over >=129 keys.

Schedule notes: PSUM copy-outs are split into half-tiles drained by ACT and
DVE in parallel (each half ~360-390ns vs the PE's 427ns per psum group, so
phase 1 stays PE-bound).  PV/Z for chunk c-1 are emitted after scores/exp of
chunk c so the ACT exp lag never stalls the PE.  Constants stream on the
gpsimd SWDGE queue, keeping the HWDGE path clear for x/W at startup.
"""

import numpy as np
import ml_dtypes
from contextlib import ExitStack

import concourse.bass as bass
import concourse.bacc as bacc
import concourse.mybir as mybir
import concourse.tile as tile
from concourse.bass_utils import run_bass_kernel_spmd

FP32 = mybir.dt.float32
F32R = mybir.dt.float32r
BF16 = mybir.dt.bfloat16
FP8 = mybir.dt.float8e4
AF = mybir.ActivationFunctionType
ALU = mybir.AluOpType
DR = mybir.MatmulPerfMode.DoubleRow

B, T, E = 8, 2048, 1024
P = 128
NE = E // P            # 8 e-tiles (contraction)
NA = NE // 2           # 4 e-pair tiles for DoubleRow
NT = T // P            # 16 t-tiles
NC = 4                 # tq chunks of 512
CH = T // NC           # 512
HC = CH // 2           # 256 (half-tile copyout split)
SCALE = 1.0 / np.sqrt(E)
WS = 16.0              # weight pre-scale before fp8 quantization
MA = -16.0             # mask matmul stationary value ...
MB = 96.0              # ... times moving value = -1536 on masked entries

E4NP = ml_dtypes.float8_e4m3fn
BFNP = ml_dtypes.bfloat16


def _build_nc():
    nc = bacc.Bacc()

    xt8_d = nc.declare_dram_parameter("xt8", [NA, P, 2, T], FP8, isOutput=False)
    xtbf_d = nc.declare_dram_parameter("xtbf", [P, NE, P], BF16, isOutput=False)
    wqk8_d = nc.declare_dram_parameter("wqk8", [2 * NE, P, NA, 2, P], FP8, isOutput=False)
    wqkbf_d = nc.declare_dram_parameter("wqkbf", [2 * NE, P, NE, P], BF16, isOutput=False)
    wv8_d = nc.declare_dram_parameter("wv8", [NA, P, 2, E], FP8, isOutput=False)
    wvbf_d = nc.declare_dram_parameter("wvbf", [P, NE, E], BF16, isOutput=False)
    bqk_d = nc.declare_dram_parameter("bqk", [P, 2 * NE], FP32, isOutput=False)
    bvrep_d = nc.declare_dram_parameter("bvrep", [P, E], FP32, isOutput=False)
    mska8_d = nc.declare_dram_parameter("mska8", [P, P], FP8, isOutput=False)
    mskb8_d = nc.declare_dram_parameter("mskb8", [P, NC, CH], FP8, isOutput=False)
    mskabf_d = nc.declare_dram_parameter("mskabf", [P, P], BF16, isOutput=False)
    mskbbf_d = nc.declare_dram_parameter("mskbbf", [P, P], BF16, isOutput=False)
    ones8_d = nc.declare_dram_parameter("ones8", [P, 2, 4], FP8, isOutput=False)
    ones0_d = nc.declare_dram_parameter("ones0", [P, 4], F32R, isOutput=False)
    y_d = nc.declare_dram_parameter("y", [T, E], FP32, isOutput=True)

    with tile.TileContext(nc) as tc:
        with ExitStack() as ctx:
            # ---- persistent pools ----
            const_pool = ctx.enter_context(tc.tile_pool(name="const", bufs=1))
            qk_pool = ctx.enter_context(tc.tile_pool(name="qk", bufs=2 * NA))
            qk0_pool = ctx.enter_context(tc.tile_pool(name="qk0", bufs=2 * NE))
            v_pool = ctx.enter_context(tc.tile_pool(name="v", bufs=NT // 2))
            psa = ctx.enter_context(tc.tile_pool(name="psa", bufs=3, space="PSUM"))

            # q pairs in qk8[0..NA), k pairs in qk8[NA..2NA)
            qk8 = [qk_pool.tile([P, 2, T], FP8, tag="qk8", name="qk8")
                   for _ in range(2 * NA)]
            qk0 = [qk0_pool.tile([P, P], BF16, tag="qk0", name="qk0")
                   for _ in range(2 * NE)]
            v8 = [v_pool.tile([P, 2, E], FP8, tag="v8", name="v8")
                  for _ in range(NT // 2)]
            v0 = const_pool.tile([P, E], F32R, tag="v0", name="v0")
            e0 = const_pool.tile([P, P], F32R, tag="e0", name="e0")

            # split copy-out: ACT takes the low half, DVE the high half
            def copyout_qk(dst_lo, dst_hi, ps, bias_ap):
                nc.scalar.activation(dst_lo, ps[:, 0:HC], AF.Identity,
                                     bias=bias_ap, scale=1.0 / WS)
                nc.vector.tensor_scalar(dst_hi, ps[:, HC:CH], 1.0 / WS, bias_ap,
                                        op0=ALU.mult, op1=ALU.add)

            def copyout_v(dst_lo, dst_hi, ps):
                nc.scalar.activation(dst_lo, ps[:, 0:HC], AF.Copy)
                nc.vector.tensor_copy(dst_hi, ps[:, HC:CH])

            # ---- phase 1: projections ----
            with ExitStack() as p1:
                xt8_pool = p1.enter_context(tc.tile_pool(name="xt8", bufs=NA))
                xtbf_pool = p1.enter_context(tc.tile_pool(name="xtbf", bufs=1))
                wqk8_pool = p1.enter_context(tc.tile_pool(name="wqk8p", bufs=6))
                wqkbf_pool = p1.enter_context(tc.tile_pool(name="wqkbfp", bufs=2 * NE))
                wv8_pool = p1.enter_context(tc.tile_pool(name="wv8p", bufs=NA))
                wvbf_pool = p1.enter_context(tc.tile_pool(name="wvbfp", bufs=1))

                # x + first weights first (critical path), sync + act queues
                xt8_sb = []
                for a2 in range(NA):
                    t_ = xt8_pool.tile([P, 2, T], FP8, tag="xt8", name="xt8")
                    q = nc.sync if a2 % 2 == 0 else nc.scalar
                    q.dma_start(t_[:], xt8_d[a2])
                    xt8_sb.append(t_)
                bqk_sb = const_pool.tile([P, 2 * NE], FP32, tag="bqk", name="bqk")
                nc.sync.dma_start(bqk_sb[:], bqk_d[:])
                xtbf_sb = xtbf_pool.tile([P, NE, P], BF16, tag="xtbf", name="xtbf")
                nc.sync.dma_start(xtbf_sb[:], xtbf_d[:])

                # constants stream on the gpsimd SWDGE queue (HWDGE stays clear)
                ones8 = const_pool.tile([P, 2, 4], FP8, tag="ones8", name="ones8")
                nc.gpsimd.dma_start(ones8[:], ones8_d[:])
                ones0 = const_pool.tile([P, 4], F32R, tag="ones0", name="ones0")
                nc.gpsimd.dma_start(ones0[:], ones0_d[:])
                mska8 = const_pool.tile([P, P], FP8, tag="mska8", name="mska8")
                nc.gpsimd.dma_start(mska8[:], mska8_d[:])
                mskb8 = const_pool.tile([P, NC, CH], FP8, tag="mskb8", name="mskb8")
                nc.gpsimd.dma_start(mskb8[:], mskb8_d[:])
                mskabf = const_pool.tile([P, P], BF16, tag="mskabf", name="mskabf")
                nc.gpsimd.dma_start(mskabf[:], mskabf_d[:])
                mskbbf = const_pool.tile([P, P], BF16, tag="mskbbf", name="mskbbf")
                nc.gpsimd.dma_start(mskbbf[:], mskbbf_d[:])
                bvrep = const_pool.tile([P, E], FP32, tag="bvrep", name="bvrep")
                nc.gpsimd.dma_start(bvrep[:], bvrep_d[:])
                wvbf_sb = wvbf_pool.tile([P, NE, E], BF16, tag="wvbf", name="wvbf")
                for e in range(NE):
                    nc.gpsimd.dma_start(wvbf_sb[:, e, :], wvbf_d[:, e, :])

                # q^T, k^T fp8 pair-tiles, f-tile by f-tile
                wv8_sb = None
                for ft in range(2 * NE):
                    wt = wqk8_pool.tile([P, NA, 2, P], FP8, tag="wqk8", name="wqk8")
                    nc.scalar.dma_start(wt[:], wqk8_d[ft])
                    if ft == 12:
                        wv8_sb = []
                        for a2 in range(NA):
                            t_ = wv8_pool.tile([P, 2, E], FP8, tag="wv8", name="wv8")
                            nc.scalar.dma_start(t_[:], wv8_d[a2])
                            wv8_sb.append(t_)
                    for tch in range(NC):
                        ps = psa.tile([P, CH], FP32, tag="ps", name="ps")
                        for a2 in range(NA):
                            nc.tensor.matmul(
                                ps[:],
                                lhsT=wt[:, a2, :, :],
                                rhs=xt8_sb[a2][:, :, tch * CH:(tch + 1) * CH],
                                start=(a2 == 0),
                                stop=(a2 == NA - 1),
                                perf_mode=DR,
                            )
                        g, i = ft // 2, ft % 2
                        c0 = tch * CH
                        copyout_qk(qk8[g][:, i, c0:c0 + HC],
                                   qk8[g][:, i, c0 + HC:c0 + CH],
                                   ps, bqk_sb[:, ft:ft + 1])

                # high-precision q0,k0 (bf16, t<128 output columns);
                # weight tiles prefetched on the sync queue
                wbf_sb = []
                for ft in range(2 * NE):
                    wbf = wqkbf_pool.tile([P, NE, P], BF16, tag="wqkbf", name="wqkbf")
                    nc.scalar.dma_start(wbf[:], wqkbf_d[ft])
                    wbf_sb.append(wbf)
                for ft in range(2 * NE):
                    ps = psa.tile([P, CH], FP32, tag="ps", name="ps")
                    for e in range(NE):
                        nc.tensor.matmul(
                            ps[:, 0:P],
                            lhsT=wbf_sb[ft][:, e, :],
                            rhs=xtbf_sb[:, e, :],
                            start=(e == 0),
                            stop=(e == NE - 1),
                        )
                    nc.scalar.activation(qk0[ft][:], ps[:, 0:P], AF.Identity,
                                         bias=bqk_sb[:, ft:ft + 1], scale=1.0)

                # high-precision scores+exp for the t<128 block
                ps = psa.tile([P, CH], FP32, tag="ps", name="ps")
                for e in range(NE):
                    nc.tensor.matmul(ps[:, 0:P], lhsT=qk0[NE + e][:],
                                     rhs=qk0[e][:],
                                     start=(e == 0), stop=False)
                nc.tensor.matmul(ps[:, 0:P], lhsT=mskabf[:], rhs=mskbbf[:],
                                 start=False, stop=True)
                nc.scalar.activation(e0[:], ps[:, 0:P], AF.Exp, scale=SCALE)

                # v in [t, e] layout: fp8 (x16) for all tiles + fp32r tile 0
                for tt in range(NT):
                    for ec in range(2):
                        ps = psa.tile([P, CH], FP32, tag="ps", name="ps")
                        for a2 in range(NA):
                            nc.tensor.matmul(
                                ps[:],
                                lhsT=xt8_sb[a2][:, :, tt * P:(tt + 1) * P],
                                rhs=wv8_sb[a2][:, :, ec * CH:(ec + 1) * CH],
                                start=(a2 == 0),
                                stop=(a2 == NA - 1),
                                perf_mode=DR,
                            )
                        c0 = ec * CH
                        copyout_v(v8[tt // 2][:, tt % 2, c0:c0 + HC],
                                  v8[tt // 2][:, tt % 2, c0 + HC:c0 + CH],
                                  ps)
                for ec in range(2):
                    ps = psa.tile([P, CH], FP32, tag="ps", name="ps")
                    for e in range(NE):
                        nc.tensor.matmul(
                            ps[:],
                            lhsT=xtbf_sb[:, e, :],
                            rhs=wvbf_sb[:, e, ec * CH:(ec + 1) * CH],
                            start=(e == 0),
                            stop=(e == NE - 1),
                        )
                    nc.scalar.activation(v0[:, ec * CH:(ec + 1) * CH], ps[:],
                                         AF.Copy)

            # ---- phases 2+3: scores+softmax+PV ----
            with ExitStack() as p2:
                exps_pool = p2.enter_context(tc.tile_pool(name="exps", bufs=16))
                y_pool = p2.enter_context(tc.tile_pool(name="yst", bufs=3))
                zr_pool = p2.enter_context(tc.tile_pool(name="zr", bufs=4))
                psy = p2.enter_context(tc.tile_pool(name="psy", bufs=3, space="PSUM"))
                psz = p2.enter_context(tc.tile_pool(name="psz", bufs=2, space="PSUM"))

                # high-precision y rows 0..127
                ps_z = psz.tile([P, 4 * NC], FP32, tag="z", name="z")
                nc.tensor.matmul(ps_z[:, 0:4], lhsT=e0[:], rhs=ones0[:],
                                 start=True, stop=True)
                zr0 = zr_pool.tile([P, 1], FP32, tag="zr", name="zr")
                nc.vector.reciprocal(zr0[:], ps_z[:, 0:1])
                y_t0 = y_pool.tile([P, E], FP32, tag="y", name="y")
                for ec in range(2):
                    ps_y = psy.tile([P, CH], FP32, tag="y", name="psy")
                    nc.tensor.matmul(ps_y[:], lhsT=e0[:],
                                     rhs=v0[:, ec * CH:(ec + 1) * CH],
                                     start=True, stop=True)
                    nc.vector.scalar_tensor_tensor(
                        y_t0[:, ec * CH:(ec + 1) * CH], ps_y[:], zr0[:],
                        bvrep[:, ec * CH:(ec + 1) * CH],
                        op0=ALU.mult, op1=ALU.add)
                    nc.sync.dma_start(y_d[0:P, ec * CH:(ec + 1) * CH],
                                      y_t0[:, ec * CH:(ec + 1) * CH])

                def scores_chunk(c):
                    n_tk = (c + 1) * (CH // P)
                    es = []
                    for tk in range(n_tk):
                        if tk % 2 == 0:
                            es.append(exps_pool.tile([P, 2, CH], FP8,
                                                     tag="es", name="es"))
                        ps = psa.tile([P, CH], FP32, tag="ps", name="ps")
                        d = tk - c * (CH // P)
                        for a2 in range(NA):
                            # mask matmul slots in before the last accumulate
                            if d >= 0 and a2 == NA - 1:
                                nc.tensor.matmul(
                                    ps[:, 0:(d + 1) * P], lhsT=mska8[:],
                                    rhs=mskb8[:, d, 0:(d + 1) * P],
                                    start=False, stop=False)
                            nc.tensor.matmul(
                                ps[:],
                                lhsT=qk8[NA + a2][:, :, tk * P:(tk + 1) * P],
                                rhs=qk8[a2][:, :, c * CH:(c + 1) * CH],
                                start=(a2 == 0),
                                stop=(a2 == NA - 1),
                                perf_mode=DR,
                            )
                        nc.scalar.activation(es[tk // 2][:, tk % 2, :], ps[:],
                                             AF.Exp, scale=SCALE)
                    return es

                def pv_chunk(c, es, js):
                    ps_z = psz.tile([P, 4 * NC], FP32, tag="z", name="z")
                    for j in js:
                        tq = c * (CH // P) + j
                        if tq == 0:
                            continue  # high-precision path above
                        nu = tq // 2 + 1  # pair-tiles; odd tail is 0 in fp8
                        for u in range(nu):
                            nc.tensor.matmul(
                                ps_z[:, 4 * j:4 * j + 4],
                                lhsT=es[u][:, :, j * P:(j + 1) * P],
                                rhs=ones8[:],
                                start=(u == 0),
                                stop=(u == nu - 1),
                                perf_mode=DR,
                            )
                        zr = zr_pool.tile([P, 1], FP32, tag="zr", name="zr")
                        nc.vector.reciprocal(zr[:], ps_z[:, 4 * j:4 * j + 1])
                        y_t = y_pool.tile([P, E], FP32, tag="y", name="y")
                        for ec in range(2):
                            ps_y = psy.tile([P, CH], FP32, tag="y", name="psy")
                            for u in range(nu):
                                nc.tensor.matmul(
                                    ps_y[:],
                                    lhsT=es[u][:, :, j * P:(j + 1) * P],
                                    rhs=v8[u][:, :, ec * CH:(ec + 1) * CH],
                                    start=(u == 0),
                                    stop=(u == nu - 1),
                                    perf_mode=DR,
                                )
                            nc.vector.scalar_tensor_tensor(
                                y_t[:, ec * CH:(ec + 1) * CH], ps_y[:], zr[:],
                                bvrep[:, ec * CH:(ec + 1) * CH],
                                op0=ALU.mult, op1=ALU.add)
                            nc.sync.dma_start(
                                y_d[tq * P:(tq + 1) * P, ec * CH:(ec + 1) * CH],
                                y_t[:, ec * CH:(ec + 1) * CH])

                # software pipeline: PV of chunk c-1 runs under scores of c
                es_prev = None
                for c in range(NC):
                    es_cur = scores_chunk(c)
                    if es_prev is not None:
                        pv_chunk(c - 1, es_prev, range(NC))
                    es_prev = es_cur
                # last chunk: cheapest tile last to shorten the tail
                pv_chunk(NC - 1, es_prev, [1, 2, 3, 0])
    nc.finalize()
    return nc


_NC_CACHE = {}


def _get_nc():
    if "nc" not in _NC_CACHE:
        _NC_CACHE["nc"] = _build_nc()
    return _NC_CACHE["nc"]


def _prep_inputs(x, W, b):
    # xt8[a2, p, i, t] = x[t, (2*a2+i)*128+p]
    xt = np.ascontiguousarray(
        x.reshape(B, T, NA, 2, P).transpose(0, 2, 4, 3, 1))       # [B,NA,P,2,T]
    xt8 = xt.astype(E4NP)
    # xtbf[p, a, t] = x[t<128, a*128+p]
    xtbf = np.ascontiguousarray(
        x[:, :P].reshape(B, P, NE, P).transpose(0, 3, 2, 1)).astype(BFNP)
    # wqk8[ft, p, a2, i, f] = 16*W[ft*128+f, (2*a2+i)*128+p]
    wqk8 = np.ascontiguousarray(
        (W[:2 * E] * WS).reshape(2 * NE, P, NA, 2, P).transpose(0, 4, 2, 3, 1)
    ).astype(E4NP)
    # wqkbf[ft, p, a, f] = W[ft*128+f, a*128+p]
    wqkbf = np.ascontiguousarray(
        W[:2 * E].reshape(2 * NE, P, NE, P).transpose(0, 3, 2, 1)).astype(BFNP)
    # wv8[a2, p, i, eo] = 16*W[2E+eo, (2*a2+i)*128+p]
    wv8 = np.ascontiguousarray(
        (W[2 * E:] * WS).reshape(E, NA, 2, P).transpose(1, 3, 2, 0)).astype(E4NP)
    # wvbf[p, a, eo] = 16*W[2E+eo, a*128+p]  (x16 so v0 matches v8 scaling)
    wvbf = np.ascontiguousarray(
        (W[2 * E:] * WS).reshape(E, NE, P).transpose(2, 1, 0)).astype(BFNP)
    # bqk[p, ft] = b[ft*128+p]
    bqk = np.ascontiguousarray(
        b[:2 * E].astype(np.float32).reshape(2 * NE, P).T).copy()
    bvrep = np.broadcast_to(b[2 * E:].astype(np.float32), (P, E)).copy()
    mska8 = (MA * np.eye(P, dtype=np.float32)).astype(E4NP)
    ii = np.arange(P)[:, None]
    jj = np.arange(CH)[None, :]
    # mskb8[r, d, j] = MB if j < d*128 + r else 0
    mskb8 = np.ascontiguousarray(np.stack(
        [np.where(jj < d * P + ii, MB, 0.0) for d in range(NC)]
    ).transpose(1, 0, 2)).astype(np.float32).astype(E4NP)
    mskabf = (MA * np.eye(P, dtype=np.float32)).astype(BFNP)
    mskbbf = np.where(jj[:, :P] < ii, MB, 0.0).astype(np.float32).astype(BFNP)
    ones8 = np.full((P, 2, 4), WS, np.float32).astype(E4NP)
    ones0 = np.full((P, 4), WS, np.float32)
    shared = {"wqk8": wqk8, "wqkbf": wqkbf, "wv8": wv8, "wvbf": wvbf,
              "bqk": bqk, "bvrep": bvrep, "mska8": mska8, "mskb8": mskb8,
              "mskabf": mskabf, "mskbbf": mskbbf, "ones8": ones8,
              "ones0": ones0}
    return [{"xt8": np.ascontiguousarray(xt8[i]),
             "xtbf": np.ascontiguousarray(xtbf[i]), **shared}
            for i in range(B)]


def run(x, W, b, **spmd_kwargs):
    nc = _get_nc()
    in_maps = _prep_inputs(np.asarray(x, np.float32), np.asarray(W, np.float32),
                           np.asarray(b, np.float32))
    res = run_bass_kernel_spmd(nc, in_maps, list(range(B)), **spmd_kwargs)
    y = np.stack([res.results[i]["y"] for i in range(B)]).astype(np.float32)
    return y, res


def kernel(x, W, b):
    y, _ = run(x, W, b)
    return y


# revision 19
# speedup vs baseline: 3.0432x; 1.1074x over previous
"""Single-head causal attention block (QKV projection + attention) on 8 TRN2 cores.

Reference computation (per batch element b, batch-sharded 1 core each):
    qkv = x[b] @ W.T + b          # [T, 3E]
    q, k, v = split(qkv)          # each [T, E]
    s = (q @ k.T) / sqrt(E), causal-masked
    y = softmax(s) @ v            # [T, E]

Shapes: B=8, T=2048, E=1024.

Speed strategy: nearly all matmul FLOPs run as fp8(e4m3) DoubleRow matmuls
(0.5 cycles/row = 2x bf16 rate).  DoubleRow contracts 256 elements per
instruction: both operands carry a pair dim in a 3D AP [128, 2, N].

Accuracy strategy (rel-err budget 2e-2, achieved ~5.8e-3 on hw):
  - W scaled x16 before e4m3 quantization (avoids subnormals); /16 folded
    into the copy-out (q,k) or into the ones/Z scaling (v path: v stored x16,
    Z computed with ones=16, so y = ps_y * (1/(16Z)) is exact).
  - q,k stored unscaled e4m3 (std ~0.58); the 1/sqrt(E) score scale is
    applied inside the Exp activation (out = exp(in*scale)).
  - causal mask applied as an extra accumulating PE matmul: (-16*I).T @
    (96*tril-indicator) = -1536 on masked entries -> exp(scale*s - 48) -> 0
    exactly in fp8.  The indicator is zero beyond column (d+1)*128, so the
    mask matmul only covers that prefix.
  - softmax row sums Z from a ones(=16)-column DoubleRow matmul; masked
    entries quantize to exactly 0 so above-diagonal pair-tiles contribute 0,
    letting every PV/Z accumulation use full pairs.
  - rows t<128 (few softmax terms -> no error averaging) are recomputed on a
    high-precision bf16/fp32r path: bf16 q0,k0,v0 projections, fp32 exp,
    fp32r PV.  This bounds the worst-case row error; rows >=128 average fp8
    noise over >=129 keys.

Schedule notes: PSUM copy-outs are split into half-tiles drained by ACT and
DVE in parallel (each half ~360-390ns vs the PE's 427ns per psum group, so
phase 1 stays PE-bound).  PV/Z for chunk c-1 are emitted after scores/exp of
chunk c so the ACT exp lag never stalls the PE.  Constants stream on the
gpsimd SWDGE queue, keeping the HWDGE path clear for x/W at startup.
"""

import numpy as np
import ml_dtypes
from contextlib import ExitStack

import concourse.bass as bass
import concourse.bacc as bacc
import concourse.mybir as mybir
import concourse.tile as tile
from concourse.bass_utils import run_bass_kernel_spmd

FP32 = mybir.dt.float32
F32R = mybir.dt.float32r
BF16 = mybir.dt.bfloat16
FP8 = mybir.dt.float8e4
AF = mybir.ActivationFunctionType
ALU = mybir.AluOpType
DR = mybir.MatmulPerfMode.DoubleRow

B, T, E = 8, 2048, 1024
P = 128
NE = E // P            # 8 e-tiles (contraction)
NA = NE // 2           # 4 e-pair tiles for DoubleRow
NT = T // P            # 16 t-tiles
NC = 4                 # tq chunks of 512
CH = T // NC           # 512
HC = CH // 2           # 256 (half-tile copyout split)
SCALE = 1.0 / np.sqrt(E)
WS = 16.0              # weight pre-scale before fp8 quantization
MA = -16.0             # mask matmul stationary value ...
MB = 96.0              # ... times moving value = -1536 on masked entries

E4NP = ml_dtypes.float8_e4m3fn
BFNP = ml_dtypes.bfloat16


def _build_nc():
    nc = bacc.Bacc()

    xt8_d = nc.declare_dram_parameter("xt8", [NA, P, 2, T], FP8, isOutput=False)
    xtbf_d = nc.declare_dram_parameter("xtbf", [P, NE, P], BF16, isOutput=False)
    wqk8_d = nc.declare_dram_parameter("wqk8", [2 * NE, P, NA, 2, P], FP8, isOutput=False)
    wqkbf_d = nc.declare_dram_parameter("wqkbf", [2 * NE, P, NE, P], BF16, isOutput=False)
    wv8_d = nc.declare_dram_parameter("wv8", [NA, P, 2, E], FP8, isOutput=False)
    wvbf_d = nc.declare_dram_parameter("wvbf", [P, NE, E], BF16, isOutput=False)
    bqk_d = nc.declare_dram_parameter("bqk", [P, 2 * NE], FP32, isOutput=False)
    bvrep_d = nc.declare_dram_parameter("bvrep", [P, E], FP32, isOutput=False)
    mska8_d = nc.declare_dram_parameter("mska8", [P, P], FP8, isOutput=False)
    mskb8_d = nc.declare_dram_parameter("mskb8", [P, NC, CH], FP8, isOutput=False)
    mskabf_d = nc.declare_dram_parameter("mskabf", [P, P], BF16, isOutput=False)
    mskbbf_d = nc.declare_dram_parameter("mskbbf", [P, P], BF16, isOutput=False)
    ones8_d = nc.declare_dram_parameter("ones8", [P, 2, 4], FP8, isOutput=False)
    ones0_d = nc.declare_dram_parameter("ones0", [P, 4], F32R, isOutput=False)
    y_d = nc.declare_dram_parameter("y", [T, E], FP32, isOutput=True)

    with tile.TileContext(nc) as tc:
        with ExitStack() as ctx:
            # ---- persistent pools ----
            const_pool = ctx.enter_context(tc.tile_pool(name="const", bufs=1))
            qk_pool = ctx.enter_context(tc.tile_pool(name="qk", bufs=2 * NA))
            qk0_pool = ctx.enter_context(tc.tile_pool(name="qk0", bufs=2 * NE))
            v_pool = ctx.enter_context(tc.tile_pool(name="v", bufs=NT // 2))
            psa = ctx.enter_context(tc.tile_pool(name="psa", bufs=4, space="PSUM"))

            # q pairs in qk8[0..NA), k pairs in qk8[NA..2NA)
            qk8 = [qk_pool.tile([P, 2, T], FP8, tag="qk8", name="qk8")
                   for _ in range(2 * NA)]
            qk0 = [qk0_pool.tile([P, P], BF16, tag="qk0", name="qk0")
                   for _ in range(2 * NE)]
            v8 = [v_pool.tile([P, 2, E], FP8, tag="v8", name="v8")
                  for _ in range(NT // 2)]
            v0 = const_pool.tile([P, E], F32R, tag="v0", name="v0")
            e0 = const_pool.tile([P, P], F32R, tag="e0", name="e0")

            # split copy-out: ACT takes the low half, DVE the high half
            def copyout_qk(dst_lo, dst_hi, ps, bias_ap):
                nc.scalar.activation(dst_lo, ps[:, 0:HC], AF.Identity,
                                     bias=bias_ap, scale=1.0 / WS)
                nc.vector.tensor_scalar(dst_hi, ps[:, HC:CH], 1.0 / WS, bias_ap,
                                        op0=ALU.mult, op1=ALU.add)

            def copyout_v(dst_lo, dst_hi, ps):
                nc.scalar.activation(dst_lo, ps[:, 0:HC], AF.Copy)
                nc.vector.tensor_copy(dst_hi, ps[:, HC:CH])

            # ---- phase 1: projections ----
            with ExitStack() as p1:
                xt8_pool = p1.enter_context(tc.tile_pool(name="xt8", bufs=NA))
                xtbf_pool = p1.enter_context(tc.tile_pool(name="xtbf", bufs=1))
                wqk8_pool = p1.enter_context(tc.tile_pool(name="wqk8p", bufs=6))
                wqkbf_pool = p1.enter_context(tc.tile_pool(name="wqkbfp", bufs=2 * NE))
                wv8_pool = p1.enter_context(tc.tile_pool(name="wv8p", bufs=NA))
                wvbf_pool = p1.enter_context(tc.tile_pool(name="wvbfp", bufs=1))

                # x + first weights first (critical path). All input DMAs go
                # on the SP (sync) and gpsimd queues so the ACT sequencer
                # stays free for activation dispatch.
                xt8_sb = []
                for a2 in range(NA):
                    t_ = xt8_pool.tile([P, 2, T], FP8, tag="xt8", name="xt8")
                    q = nc.sync if a2 % 2 == 0 else nc.gpsimd
                    q.dma_start(t_[:], xt8_d[a2])
                    xt8_sb.append(t_)
                bqk_sb = const_pool.tile([P, 2 * NE], FP32, tag="bqk", name="bqk")
                nc.sync.dma_start(bqk_sb[:], bqk_d[:])
                xtbf_sb = xtbf_pool.tile([P, NE, P], BF16, tag="xtbf", name="xtbf")
                nc.sync.dma_start(xtbf_sb[:], xtbf_d[:])

                # constants stream on the gpsimd SWDGE queue (HWDGE stays clear)
                ones8 = const_pool.tile([P, 2, 4], FP8, tag="ones8", name="ones8")
                nc.gpsimd.dma_start(ones8[:], ones8_d[:])
                ones0 = const_pool.tile([P, 4], F32R, tag="ones0", name="ones0")
                nc.gpsimd.dma_start(ones0[:], ones0_d[:])
                mska8 = const_pool.tile([P, P], FP8, tag="mska8", name="mska8")
                nc.gpsimd.dma_start(mska8[:], mska8_d[:])
                mskb8 = const_pool.tile([P, NC, CH], FP8, tag="mskb8", name="mskb8")
                nc.gpsimd.dma_start(mskb8[:], mskb8_d[:])
                mskabf = const_pool.tile([P, P], BF16, tag="mskabf", name="mskabf")
                nc.gpsimd.dma_start(mskabf[:], mskabf_d[:])
                mskbbf = const_pool.tile([P, P], BF16, tag="mskbbf", name="mskbbf")
                nc.gpsimd.dma_start(mskbbf[:], mskbbf_d[:])
                bvrep = const_pool.tile([P, E], FP32, tag="bvrep", name="bvrep")
                nc.gpsimd.dma_start(bvrep[:], bvrep_d[:])
                wvbf_sb = wvbf_pool.tile([P, NE, E], BF16, tag="wvbf", name="wvbf")
                for e in range(NE):
                    nc.gpsimd.dma_start(wvbf_sb[:, e, :], wvbf_d[:, e, :])

                # q^T, k^T fp8 pair-tiles, f-tile by f-tile
                wv8_sb = None
                for ft in range(2 * NE):
                    wt = wqk8_pool.tile([P, NA, 2, P], FP8, tag="wqk8", name="wqk8")
                    nc.sync.dma_start(wt[:], wqk8_d[ft])
                    if ft == 12:
                        wv8_sb = []
                        for a2 in range(NA):
                            t_ = wv8_pool.tile([P, 2, E], FP8, tag="wv8", name="wv8")
                            nc.sync.dma_start(t_[:], wv8_d[a2])
                            wv8_sb.append(t_)
                    for tch in range(NC):
                        ps = psa.tile([P, CH], FP32, tag="ps", name="ps")
                        for a2 in range(NA):
                            nc.tensor.matmul(
                                ps[:],
                                lhsT=wt[:, a2, :, :],
                                rhs=xt8_sb[a2][:, :, tch * CH:(tch + 1) * CH],
                                start=(a2 == 0),
                                stop=(a2 == NA - 1),
                                perf_mode=DR,
                            )
                        g, i = ft // 2, ft % 2
                        c0 = tch * CH
                        copyout_qk(qk8[g][:, i, c0:c0 + HC],
                                   qk8[g][:, i, c0 + HC:c0 + CH],
                                   ps, bqk_sb[:, ft:ft + 1])

                # high-precision q0,k0 (bf16, t<128 output columns);
                # weight tiles prefetched on the sync queue
                wbf_sb = []
                for ft in range(2 * NE):
                    wbf = wqkbf_pool.tile([P, NE, P], BF16, tag="wqkbf", name="wqkbf")
                    nc.sync.dma_start(wbf[:], wqkbf_d[ft])
                    wbf_sb.append(wbf)
                for ft in range(2 * NE):
                    ps = psa.tile([P, CH], FP32, tag="ps", name="ps")
                    for e in range(NE):
                        nc.tensor.matmul(
                            ps[:, 0:P],
                            lhsT=wbf_sb[ft][:, e, :],
                            rhs=xtbf_sb[:, e, :],
                            start=(e == 0),
                            stop=(e == NE - 1),
                        )
                    nc.scalar.activation(qk0[ft][:], ps[:, 0:P], AF.Identity,
                                         bias=bqk_sb[:, ft:ft + 1], scale=1.0)

                # high-precision scores+exp for the t<128 block
                ps = psa.tile([P, CH], FP32, tag="ps", name="ps")
                for e in range(NE):
                    nc.tensor.matmul(ps[:, 0:P], lhsT=qk0[NE + e][:],
                                     rhs=qk0[e][:],
                                     start=(e == 0), stop=False)
                nc.tensor.matmul(ps[:, 0:P], lhsT=mskabf[:], rhs=mskbbf[:],
                                 start=False, stop=True)
                nc.scalar.activation(e0[:], ps[:, 0:P], AF.Exp, scale=SCALE)

                # v in [t, e] layout: fp8 (x16) for all tiles + fp32r tile 0
                for tt in range(NT):
                    for ec in range(2):
                        ps = psa.tile([P, CH], FP32, tag="ps", name="ps")
                        for a2 in range(NA):
                            nc.tensor.matmul(
                                ps[:],
                                lhsT=xt8_sb[a2][:, :, tt * P:(tt + 1) * P],
                                rhs=wv8_sb[a2][:, :, ec * CH:(ec + 1) * CH],
                                start=(a2 == 0),
                                stop=(a2 == NA - 1),
                                perf_mode=DR,
                            )
                        c0 = ec * CH
                        copyout_v(v8[tt // 2][:, tt % 2, c0:c0 + HC],
                                  v8[tt // 2][:, tt % 2, c0 + HC:c0 + CH],
                                  ps)
                for ec in range(2):
                    ps = psa.tile([P, CH], FP32, tag="ps", name="ps")
                    for e in range(NE):
                        nc.tensor.matmul(
                            ps[:],
                            lhsT=xtbf_sb[:, e, :],
                            rhs=wvbf_sb[:, e, ec * CH:(ec + 1) * CH],
                            start=(e == 0),
                            stop=(e == NE - 1),
                        )
                    nc.scalar.activation(v0[:, ec * CH:(ec + 1) * CH], ps[:],
                                         AF.Copy)

            # ---- phases 2+3: scores+softmax+PV ----
            with ExitStack() as p2:
                exps_pool = p2.enter_context(tc.tile_pool(name="exps", bufs=16))
                y_pool = p2.enter_context(tc.tile_pool(name="yst", bufs=3))
                zr_pool = p2.enter_context(tc.tile_pool(name="zr", bufs=4))
                psy = p2.enter_context(tc.tile_pool(name="psy", bufs=3, space="PSUM"))
                psz = p2.enter_context(tc.tile_pool(name="psz", bufs=1, space="PSUM"))

                # high-precision y rows 0..127
                ps_z = psz.tile([P, 4 * NC], FP32, tag="z", name="z")
                nc.tensor.matmul(ps_z[:, 0:4], lhsT=e0[:], rhs=ones0[:],
                                 start=True, stop=True)
                zr0 = zr_pool.tile([P, 1], FP32, tag="zr", name="zr")
                nc.vector.reciprocal(zr0[:], ps_z[:, 0:1])
                y_t0 = y_pool.tile([P, E], FP32, tag="y", name="y")
                for ec in range(2):
                    ps_y = psy.tile([P, CH], FP32, tag="y", name="psy")
                    nc.tensor.matmul(ps_y[:], lhsT=e0[:],
                                     rhs=v0[:, ec * CH:(ec + 1) * CH],
                                     start=True, stop=True)
                    nc.vector.scalar_tensor_tensor(
                        y_t0[:, ec * CH:(ec + 1) * CH], ps_y[:], zr0[:],
                        bvrep[:, ec * CH:(ec + 1) * CH],
                        op0=ALU.mult, op1=ALU.add)
                    nc.sync.dma_start(y_d[0:P, ec * CH:(ec + 1) * CH],
                                      y_t0[:, ec * CH:(ec + 1) * CH])

                def scores_chunk(c):
                    n_tk = (c + 1) * (CH // P)
                    es = []
                    for tk in range(n_tk):
                        if tk % 2 == 0:
                            es.append(exps_pool.tile([P, 2, CH], FP8,
                                                     tag="es", name="es"))
                        ps = psa.tile([P, CH], FP32, tag="ps", name="ps")
                        d = tk - c * (CH // P)
                        for a2 in range(NA):
                            # mask matmul slots in before the last accumulate
                            if d >= 0 and a2 == NA - 1:
                                nc.tensor.matmul(
                                    ps[:, 0:(d + 1) * P], lhsT=mska8[:],
                                    rhs=mskb8[:, d, 0:(d + 1) * P],
                                    start=False, stop=False)
                            nc.tensor.matmul(
                                ps[:],
                                lhsT=qk8[NA + a2][:, :, tk * P:(tk + 1) * P],
                                rhs=qk8[a2][:, :, c * CH:(c + 1) * CH],
                                start=(a2 == 0),
                                stop=(a2 == NA - 1),
                                perf_mode=DR,
                            )
                        nc.scalar.activation(es[tk // 2][:, tk % 2, :], ps[:],
                                             AF.Exp, scale=SCALE)
                    return es

                def pv_chunk(c, es, js):
                    ps_z = psz.tile([P, 4 * NC], FP32, tag="z", name="z")
                    for j in js:
                        tq = c * (CH // P) + j
                        if tq == 0:
                            continue  # high-precision path above
                        nu = tq // 2 + 1  # pair-tiles; odd tail is 0 in fp8
                        for u in range(nu):
                            nc.tensor.matmul(
                                ps_z[:, 4 * j:4 * j + 4],
                                lhsT=es[u][:, :, j * P:(j + 1) * P],
                                rhs=ones8[:],
                                start=(u == 0),
                                stop=(u == nu - 1),
                                perf_mode=DR,
                            )
                        zr = zr_pool.tile([P, 1], FP32, tag="zr", name="zr")
                        nc.vector.reciprocal(zr[:], ps_z[:, 4 * j:4 * j + 1])
                        y_t = y_pool.tile([P, E], FP32, tag="y", name="y")
                        for ec in range(2):
                            ps_y = psy.tile([P, CH], FP32, tag="y", name="psy")
                            for u in range(nu):
                                nc.tensor.matmul(
                                    ps_y[:],
                                    lhsT=es[u][:, :, j * P:(j + 1) * P],
                                    rhs=v8[u][:, :, ec * CH:(ec + 1) * CH],
                                    start=(u == 0),
                                    stop=(u == nu - 1),
                                    perf_mode=DR,
                                )
                            nc.vector.scalar_tensor_tensor(
                                y_t[:, ec * CH:(ec + 1) * CH], ps_y[:], zr[:],
                                bvrep[:, ec * CH:(ec + 1) * CH],
                                op0=ALU.mult, op1=ALU.add)
                            nc.sync.dma_start(
                                y_d[tq * P:(tq + 1) * P, ec * CH:(ec + 1) * CH],
                                y_t[:, ec * CH:(ec + 1) * CH])

                # software pipeline: PV of chunk c-1 runs under scores of c
                es_prev = None
                for c in range(NC):
                    es_cur = scores_chunk(c)
                    if es_prev is not None:
                        pv_chunk(c - 1, es_prev, range(NC))
                    es_prev = es_cur
                # last chunk: cheapest tile last to shorten the tail
                pv_chunk(NC - 1, es_prev, [1, 2, 3, 0])
    nc.finalize()
    return nc


_NC_CACHE = {}


def _get_nc():
    if "nc" not in _NC_CACHE:
        _NC_CACHE["nc"] = _build_nc()
    return _NC_CACHE["nc"]


def _prep_inputs(x, W, b):
    # xt8[a2, p, i, t] = x[t, (2*a2+i)*128+p]
    xt = np.ascontiguousarray(
        x.reshape(B, T, NA, 2, P).transpose(0, 2, 4, 3, 1))       # [B,NA,P,2,T]
    xt8 = xt.astype(E4NP)
    # xtbf[p, a, t] = x[t<128, a*128+p]
    xtbf = np.ascontiguousarray(
        x[:, :P].reshape(B, P, NE, P).transpose(0, 3, 2, 1)).astype(BFNP)
    # wqk8[ft, p, a2, i, f] = 16*W[ft*128+f, (2*a2+i)*128+p]
    wqk8 = np.ascontiguousarray(
        (W[:2 * E] * WS).reshape(2 * NE, P, NA, 2, P).transpose(0, 4, 2, 3, 1)
    ).astype(E4NP)
    # wqkbf[ft, p, a, f] = W[ft*128+f, a*128+p]
    wqkbf = np.ascontiguousarray(
        W[:2 * E].reshape(2 * NE, P, NE, P).transpose(0, 3, 2, 1)).astype(BFNP)
    # wv8[a2, p, i, eo] = 16*W[2E+eo, (2*a2+i)*128+p]
    wv8 = np.ascontiguousarray(
        (W[2 * E:] * WS).reshape(E, NA, 2, P).transpose(1, 3, 2, 0)).astype(E4NP)
    # wvbf[p, a, eo] = 16*W[2E+eo, a*128+p]  (x16 so v0 matches v8 scaling)
    wvbf = np.ascontiguousarray(
        (W[2 * E:] * WS).reshape(E, NE, P).transpose(2, 1, 0)).astype(BFNP)
    # bqk[p, ft] = b[ft*128+p]
    bqk = np.ascontiguousarray(
        b[:2 * E].astype(np.float32).reshape(2 * NE, P).T).copy()
    bvrep = np.broadcast_to(b[2 * E:].astype(np.float32), (P, E)).copy()
    mska8 = (MA * np.eye(P, dtype=np.float32)).astype(E4NP)
    ii = np.arange(P)[:, None]
    jj = np.arange(CH)[None, :]
    # mskb8[r, d, j] = MB if j < d*128 + r else 0
    mskb8 = np.ascontiguousarray(np.stack(
        [np.where(jj < d * P + ii, MB, 0.0) for d in range(NC)]
    ).transpose(1, 0, 2)).astype(np.float32).astype(E4NP)
    mskabf = (MA * np.eye(P, dtype=np.float32)).astype(BFNP)
    mskbbf = np.where(jj[:, :P] < ii, MB, 0.0).astype(np.float32).astype(BFNP)
    ones8 = np.full((P, 2, 4), WS, np.float32).astype(E4NP)
    ones0 = np.full((P, 4), WS, np.float32)
    shared = {"wqk8": wqk8, "wqkbf": wqkbf, "wv8": wv8, "wvbf": wvbf,
              "bqk": bqk, "bvrep": bvrep, "mska8": mska8, "mskb8": mskb8,
              "mskabf": mskabf, "mskbbf": mskbbf, "ones8": ones8,
              "ones0": ones0}
    return [{"xt8": np.ascontiguousarray(xt8[i]),
             "xtbf": np.ascontiguousarray(xtbf[i]), **shared}
            for i in range(B)]


def run(x, W, b, **spmd_kwargs):
    nc = _get_nc()
    in_maps = _prep_inputs(np.asarray(x, np.float32), np.asarray(W, np.float32),
                           np.asarray(b, np.float32))
    res = run_bass_kernel_spmd(nc, in_maps, list(range(B)), **spmd_kwargs)
    y = np.stack([res.results[i]["y"] for i in range(B)]).astype(np.float32)
    return y, res


def kernel(x, W, b):
    y, _ = run(x, W, b)
    return y


# revision 26
# speedup vs baseline: 3.2905x; 1.0812x over previous
"""Single-head causal attention block (QKV projection + attention) on 8 TRN2 cores.

Reference computation (per batch element b, batch-sharded 1 core each):
    qkv = x[b] @ W.T + b          # [T, 3E]
    q, k, v = split(qkv)          # each [T, E]
    s = (q @ k.T) / sqrt(E), causal-masked
    y = softmax(s) @ v            # [T, E]

Shapes: B=8, T=2048, E=1024.

Speed strategy: nearly all matmul FLOPs run as fp8(e4m3) DoubleRow matmuls
(0.5 cycles/row = 2x bf16 rate).  DoubleRow contracts 256 elements per
instruction: both operands carry a pair dim in a 3D AP [128, 2, N].

Accuracy strategy (rel-err budget 2e-2, achieved ~5.8e-3 on hw):
  - W scaled x16 before e4m3 quantization (avoids subnormals); /16 folded
    into the copy-out (q,k) or into the ones/Z scaling (v path: v stored x16,
    Z computed with ones=16, so y = ps_y * (1/(16Z)) is exact).
  - q,k stored unscaled e4m3 (std ~0.58); the 1/sqrt(E) score scale is
    applied inside the Exp activation (out = exp(in*scale)).
  - causal mask applied as an extra accumulating PE matmul: (-16*I).T @
    (96*tril-indicator) = -1536 on masked entries -> exp(scale*s - 48) -> 0
    exactly in fp8.  The indicator is zero beyond column (d+1)*128, so the
    mask matmul only covers that prefix.
  - softmax row sums Z from a ones(=16)-column DoubleRow matmul; masked
    entries quantize to exactly 0 so above-diagonal pair-tiles contribute 0,
    letting every PV/Z accumulation use full pairs.
  - rows t<128 (few softmax terms -> no error averaging) are recomputed on a
    high-precision bf16/fp32r path: bf16 q0,k0,v0 projections, fp32 exp,
    fp32r PV.  This bounds the worst-case row error; rows >=128 average fp8
    noise over >=129 keys.

Schedule notes: PSUM copy-outs are split into half-tiles drained by ACT and
DVE in parallel (each half ~360-390ns vs the PE's 427ns per psum group, so
phase 1 stays PE-bound).  PV/Z for chunk c-1 are emitted after scores/exp of
chunk c so the ACT exp lag never stalls the PE.  Constants stream on the
gpsimd SWDGE queue, keeping the HWDGE path clear for x/W at startup.
"""

import numpy as np
import ml_dtypes
from contextlib import ExitStack

import concourse.bass as bass
import concourse.bacc as bacc
import concourse.mybir as mybir
import concourse.tile as tile
from concourse.bass_utils import run_bass_kernel_spmd

FP32 = mybir.dt.float32
F32R = mybir.dt.float32r
BF16 = mybir.dt.bfloat16
FP8 = mybir.dt.float8e4
AF = mybir.ActivationFunctionType
ALU = mybir.AluOpType
DR = mybir.MatmulPerfMode.DoubleRow

B, T, E = 8, 2048, 1024
P = 128
NE = E // P            # 8 e-tiles (contraction)
NA = NE // 2           # 4 e-pair tiles for DoubleRow
NT = T // P            # 16 t-tiles
NC = 4                 # tq chunks of 512
CH = T // NC           # 512
HC = CH // 2           # 256 (half-tile copyout split)
SCALE = 1.0 / np.sqrt(E)
WS = 16.0              # weight pre-scale before fp8 quantization
MA = -16.0             # mask matmul stationary value ...
MB = 96.0              # ... times moving value = -1536 on masked entries

E4NP = ml_dtypes.float8_e4m3fn
BFNP = ml_dtypes.bfloat16


def _build_nc():
    nc = bacc.Bacc()

    xt8_d = nc.declare_dram_parameter("xt8", [NA, P, 2, T], FP8, isOutput=False)
    xtbf_d = nc.declare_dram_parameter("xtbf", [P, NE, P], BF16, isOutput=False)
    wqk8_d = nc.declare_dram_parameter("wqk8", [2 * NE, P, NA, 2, P], FP8, isOutput=False)
    wqkbf_d = nc.declare_dram_parameter("wqkbf", [2 * NE, P, NE, P], BF16, isOutput=False)
    wv8_d = nc.declare_dram_parameter("wv8", [NA, P, 2, E], FP8, isOutput=False)
    wvbf_d = nc.declare_dram_parameter("wvbf", [P, NE, E], BF16, isOutput=False)
    bqk_d = nc.declare_dram_parameter("bqk", [P, 2 * NE], FP32, isOutput=False)
    bvrep_d = nc.declare_dram_parameter("bvrep", [P, E], FP32, isOutput=False)
    mska8_d = nc.declare_dram_parameter("mska8", [P, P], FP8, isOutput=False)
    mskb8_d = nc.declare_dram_parameter("mskb8", [P, P], FP8, isOutput=False)
    mskabf_d = nc.declare_dram_parameter("mskabf", [P, P], BF16, isOutput=False)
    mskbbf_d = nc.declare_dram_parameter("mskbbf", [P, P], BF16, isOutput=False)
    ones8_d = nc.declare_dram_parameter("ones8", [P, 2, 4], FP8, isOutput=False)
    ones0_d = nc.declare_dram_parameter("ones0", [P, 4], F32R, isOutput=False)
    y_d = nc.declare_dram_parameter("y", [T, E], FP32, isOutput=True)

    with tile.TileContext(nc) as tc:
        with ExitStack() as ctx:
            # ---- persistent pools ----
            const_pool = ctx.enter_context(tc.tile_pool(name="const", bufs=1))
            qk_pool = ctx.enter_context(tc.tile_pool(name="qk", bufs=2 * NA))
            qk0_pool = ctx.enter_context(tc.tile_pool(name="qk0", bufs=2 * NE))
            v_pool = ctx.enter_context(tc.tile_pool(name="v", bufs=NT // 2))
            psa = ctx.enter_context(tc.tile_pool(name="psa", bufs=4, space="PSUM"))

            # q pairs in qk8[0..NA), k pairs in qk8[NA..2NA)
            qk8 = [qk_pool.tile([P, 2, T], FP8, tag="qk8", name="qk8")
                   for _ in range(2 * NA)]
            qk0 = [qk0_pool.tile([P, P], BF16, tag="qk0", name="qk0")
                   for _ in range(2 * NE)]
            v8 = [v_pool.tile([P, 2, E], FP8, tag="v8", name="v8")
                  for _ in range(NT // 2)]
            v0 = const_pool.tile([P, E], F32R, tag="v0", name="v0")
            e0 = const_pool.tile([P, P], F32R, tag="e0", name="e0")

            # split copy-out: ACT takes the low half, DVE the high half
            def copyout_qk(dst_lo, dst_hi, ps, bias_ap):
                nc.scalar.activation(dst_lo, ps[:, 0:HC], AF.Identity,
                                     bias=bias_ap, scale=1.0 / WS)
                nc.vector.tensor_scalar(dst_hi, ps[:, HC:CH], 1.0 / WS, bias_ap,
                                        op0=ALU.mult, op1=ALU.add)

            def copyout_v(dst_lo, dst_hi, ps):
                nc.scalar.activation(dst_lo, ps[:, 0:HC], AF.Copy)
                nc.vector.tensor_copy(dst_hi, ps[:, HC:CH])

            # ---- phase 1: projections ----
            with ExitStack() as p1:
                xt8_pool = p1.enter_context(tc.tile_pool(name="xt8", bufs=NA))
                xtbf_pool = p1.enter_context(tc.tile_pool(name="xtbf", bufs=1))
                wqk8_pool = p1.enter_context(tc.tile_pool(name="wqk8p", bufs=6))
                wqkbf_pool = p1.enter_context(tc.tile_pool(name="wqkbfp", bufs=2 * NE))
                wv8_pool = p1.enter_context(tc.tile_pool(name="wv8p", bufs=NA))
                wvbf_pool = p1.enter_context(tc.tile_pool(name="wvbfp", bufs=1))

                # x + first weights first (critical path). All input DMAs go
                # on the SP (sync) and gpsimd queues so the ACT sequencer
                # stays free for activation dispatch.
                wt_pre = []
                wt0 = wqk8_pool.tile([P, NA, 2, P], FP8, tag="wqk8", name="wqk8")
                nc.sync.dma_start(wt0[:], wqk8_d[0])
                wt_pre.append(wt0)
                xt8_sb = []
                for a2 in range(NA):
                    t_ = xt8_pool.tile([P, 2, T], FP8, tag="xt8", name="xt8")
                    q = nc.sync if a2 % 2 == 0 else nc.gpsimd
                    q.dma_start(t_[:], xt8_d[a2])
                    xt8_sb.append(t_)
                bqk_sb = const_pool.tile([P, 2 * NE], FP32, tag="bqk", name="bqk")
                nc.sync.dma_start(bqk_sb[:], bqk_d[:])
                wt1 = wqk8_pool.tile([P, NA, 2, P], FP8, tag="wqk8", name="wqk8")
                nc.sync.dma_start(wt1[:], wqk8_d[1])
                wt_pre.append(wt1)

                # constants stream on the gpsimd SWDGE queue (HWDGE stays clear)
                ones8 = const_pool.tile([P, 2, 4], FP8, tag="ones8", name="ones8")
                nc.gpsimd.dma_start(ones8[:], ones8_d[:])
                ones0 = const_pool.tile([P, 4], F32R, tag="ones0", name="ones0")
                nc.gpsimd.dma_start(ones0[:], ones0_d[:])
                mska8 = const_pool.tile([P, P], FP8, tag="mska8", name="mska8")
                nc.gpsimd.dma_start(mska8[:], mska8_d[:])
                mskb8 = const_pool.tile([P, P], FP8, tag="mskb8", name="mskb8")
                nc.gpsimd.dma_start(mskb8[:], mskb8_d[:])
                mskabf = const_pool.tile([P, P], BF16, tag="mskabf", name="mskabf")
                nc.gpsimd.dma_start(mskabf[:], mskabf_d[:])
                mskbbf = const_pool.tile([P, P], BF16, tag="mskbbf", name="mskbbf")
                nc.gpsimd.dma_start(mskbbf[:], mskbbf_d[:])
                bvrep = const_pool.tile([P, E], FP32, tag="bvrep", name="bvrep")
                nc.gpsimd.dma_start(bvrep[:], bvrep_d[:])
                wvbf_sb = wvbf_pool.tile([P, NE, E], BF16, tag="wvbf", name="wvbf")
                for e in range(NE):
                    nc.gpsimd.dma_start(wvbf_sb[:, e, :], wvbf_d[:, e, :])

                # q^T, k^T fp8 pair-tiles, f-tile by f-tile
                wv8_sb = None
                for ft in range(2 * NE):
                    if ft < 2:
                        wt = wt_pre[ft]
                    else:
                        wt = wqk8_pool.tile([P, NA, 2, P], FP8, tag="wqk8", name="wqk8")
                        nc.sync.dma_start(wt[:], wqk8_d[ft])
                    if ft == 12:
                        wv8_sb = []
                        for a2 in range(NA):
                            t_ = wv8_pool.tile([P, 2, E], FP8, tag="wv8", name="wv8")
                            nc.sync.dma_start(t_[:], wv8_d[a2])
                            wv8_sb.append(t_)
                    for tch in range(NC):
                        ps = psa.tile([P, CH], FP32, tag="ps", name="ps")
                        for a2 in range(NA):
                            nc.tensor.matmul(
                                ps[:],
                                lhsT=wt[:, a2, :, :],
                                rhs=xt8_sb[a2][:, :, tch * CH:(tch + 1) * CH],
                                start=(a2 == 0),
                                stop=(a2 == NA - 1),
                                perf_mode=DR,
                            )
                        g, i = ft // 2, ft % 2
                        c0 = tch * CH
                        copyout_qk(qk8[g][:, i, c0:c0 + HC],
                                   qk8[g][:, i, c0 + HC:c0 + CH],
                                   ps, bqk_sb[:, ft:ft + 1])

                # high-precision q0,k0 (bf16, t<128 output columns);
                # weight tiles prefetched on the sync queue
                xtbf_sb = xtbf_pool.tile([P, NE, P], BF16, tag="xtbf", name="xtbf")
                nc.sync.dma_start(xtbf_sb[:], xtbf_d[:])
                wbf_sb = []
                for ft in range(2 * NE):
                    wbf = wqkbf_pool.tile([P, NE, P], BF16, tag="wqkbf", name="wqkbf")
                    nc.sync.dma_start(wbf[:], wqkbf_d[ft])
                    wbf_sb.append(wbf)
                for ft in range(2 * NE):
                    ps = psa.tile([P, CH], FP32, tag="ps", name="ps")
                    for e in range(NE):
                        nc.tensor.matmul(
                            ps[:, 0:P],
                            lhsT=wbf_sb[ft][:, e, :],
                            rhs=xtbf_sb[:, e, :],
                            start=(e == 0),
                            stop=(e == NE - 1),
                        )
                    nc.scalar.activation(qk0[ft][:], ps[:, 0:P], AF.Identity,
                                         bias=bqk_sb[:, ft:ft + 1], scale=1.0)

                # high-precision scores+exp for the t<128 block
                ps = psa.tile([P, CH], FP32, tag="ps", name="ps")
                for e in range(NE):
                    nc.tensor.matmul(ps[:, 0:P], lhsT=qk0[NE + e][:],
                                     rhs=qk0[e][:],
                                     start=(e == 0), stop=False)
                nc.tensor.matmul(ps[:, 0:P], lhsT=mskabf[:], rhs=mskbbf[:],
                                 start=False, stop=True)
                nc.scalar.activation(e0[:], ps[:, 0:P], AF.Exp, scale=SCALE)

                # v in [t, e] layout: fp8 (x16) for all tiles + fp32r tile 0
                for tt in range(NT):
                    for ec in range(2):
                        ps = psa.tile([P, CH], FP32, tag="ps", name="ps")
                        for a2 in range(NA):
                            nc.tensor.matmul(
                                ps[:],
                                lhsT=xt8_sb[a2][:, :, tt * P:(tt + 1) * P],
                                rhs=wv8_sb[a2][:, :, ec * CH:(ec + 1) * CH],
                                start=(a2 == 0),
                                stop=(a2 == NA - 1),
                                perf_mode=DR,
                            )
                        c0 = ec * CH
                        copyout_v(v8[tt // 2][:, tt % 2, c0:c0 + HC],
                                  v8[tt // 2][:, tt % 2, c0 + HC:c0 + CH],
                                  ps)
                for ec in range(2):
                    ps = psa.tile([P, CH], FP32, tag="ps", name="ps")
                    for e in range(NE):
                        nc.tensor.matmul(
                            ps[:],
                            lhsT=xtbf_sb[:, e, :],
                            rhs=wvbf_sb[:, e, ec * CH:(ec + 1) * CH],
                            start=(e == 0),
                            stop=(e == NE - 1),
                        )
                    nc.scalar.activation(v0[:, ec * CH:(ec + 1) * CH], ps[:],
                                         AF.Copy)

            # ---- phases 2+3: scores+softmax+PV ----
            with ExitStack() as p2:
                exps_pool = p2.enter_context(tc.tile_pool(name="exps", bufs=16))
                y_pool = p2.enter_context(tc.tile_pool(name="yst", bufs=3))
                zr_pool = p2.enter_context(tc.tile_pool(name="zr", bufs=4))
                psy = p2.enter_context(tc.tile_pool(name="psy", bufs=3, space="PSUM"))
                psz = p2.enter_context(tc.tile_pool(name="psz", bufs=1, space="PSUM"))

                # high-precision y rows 0..127
                ps_z = psz.tile([P, 4 * NC], FP32, tag="z", name="z")
                nc.tensor.matmul(ps_z[:, 0:4], lhsT=e0[:], rhs=ones0[:],
                                 start=True, stop=True)
                zr0 = zr_pool.tile([P, 1], FP32, tag="zr", name="zr")
                nc.vector.reciprocal(zr0[:], ps_z[:, 0:1])
                y_t0 = y_pool.tile([P, E], FP32, tag="y", name="y")
                for ec in range(2):
                    ps_y = psy.tile([P, CH], FP32, tag="y", name="psy")
                    nc.tensor.matmul(ps_y[:], lhsT=e0[:],
                                     rhs=v0[:, ec * CH:(ec + 1) * CH],
                                     start=True, stop=True)
                    nc.vector.scalar_tensor_tensor(
                        y_t0[:, ec * CH:(ec + 1) * CH], ps_y[:], zr0[:],
                        bvrep[:, ec * CH:(ec + 1) * CH],
                        op0=ALU.mult, op1=ALU.add)
                    nc.sync.dma_start(y_d[0:P, ec * CH:(ec + 1) * CH],
                                      y_t0[:, ec * CH:(ec + 1) * CH])

                def scores_chunk(c):
                    n_tk = (c + 1) * (CH // P)
                    es = []
                    for tk in range(n_tk):
                        if tk % 2 == 0:
                            es.append(exps_pool.tile([P, 2, CH], FP8,
                                                     tag="es", name="es"))
                        ps = psa.tile([P, CH], FP32, tag="ps", name="ps")
                        d = tk - c * (CH // P)
                        # columns < d*128 are fully above-diagonal: skip them
                        lo = max(d, 0) * P
                        for a2 in range(NA):
                            # triangular mask on the diagonal sub-block only
                            if d >= 0 and a2 == NA - 1:
                                nc.tensor.matmul(
                                    ps[:, lo:lo + P], lhsT=mska8[:],
                                    rhs=mskb8[:],
                                    start=False, stop=False)
                            nc.tensor.matmul(
                                ps[:, lo:CH],
                                lhsT=qk8[NA + a2][:, :, tk * P:(tk + 1) * P],
                                rhs=qk8[a2][:, :, c * CH + lo:(c + 1) * CH],
                                start=(a2 == 0),
                                stop=(a2 == NA - 1),
                                perf_mode=DR,
                            )
                        nc.scalar.activation(es[tk // 2][:, tk % 2, lo:CH],
                                             ps[:, lo:CH], AF.Exp, scale=SCALE)
                    # the even-j PV/Z pair tails read tile j+1 at column
                    # block jj as zeros; those regions were skipped above,
                    # so zero them explicitly (gpsimd: SBUF-only, idle)
                    for jj in (0, 2):
                        tq = c * (CH // P) + jj
                        if tq == 0:
                            continue
                        tk = c * (CH // P) + jj + 1
                        nc.gpsimd.memset(
                            es[tk // 2][:, tk % 2, jj * P:(jj + 1) * P], 0.0)
                    return es

                def pv_chunk(c, es, js):
                    ps_z = psz.tile([P, 4 * NC], FP32, tag="z", name="z")
                    for j in js:
                        tq = c * (CH // P) + j
                        if tq == 0:
                            continue  # high-precision path above
                        nu = tq // 2 + 1  # pair-tiles; odd tail is 0 in fp8
                        for u in range(nu):
                            nc.tensor.matmul(
                                ps_z[:, 4 * j:4 * j + 4],
                                lhsT=es[u][:, :, j * P:(j + 1) * P],
                                rhs=ones8[:],
                                start=(u == 0),
                                stop=(u == nu - 1),
                                perf_mode=DR,
                            )
                        zr = zr_pool.tile([P, 1], FP32, tag="zr", name="zr")
                        nc.vector.reciprocal(zr[:], ps_z[:, 4 * j:4 * j + 1])
                        y_t = y_pool.tile([P, E], FP32, tag="y", name="y")
                        for ec in range(2):
                            ps_y = psy.tile([P, CH], FP32, tag="y", name="psy")
                            for u in range(nu):
                                nc.tensor.matmul(
                                    ps_y[:],
                                    lhsT=es[u][:, :, j * P:(j + 1) * P],
                                    rhs=v8[u][:, :, ec * CH:(ec + 1) * CH],
                                    start=(u == 0),
                                    stop=(u == nu - 1),
                                    perf_mode=DR,
                                )
                            nc.vector.scalar_tensor_tensor(
                                y_t[:, ec * CH:(ec + 1) * CH], ps_y[:], zr[:],
                                bvrep[:, ec * CH:(ec + 1) * CH],
                                op0=ALU.mult, op1=ALU.add)
                            nc.sync.dma_start(
                                y_d[tq * P:(tq + 1) * P, ec * CH:(ec + 1) * CH],
                                y_t[:, ec * CH:(ec + 1) * CH])

                # software pipeline: PV of chunk c-1 runs under scores of c
                es_prev = None
                for c in range(NC):
                    es_cur = scores_chunk(c)
                    if es_prev is not None:
                        pv_chunk(c - 1, es_prev, range(NC))
                    es_prev = es_cur
                # last chunk: cheapest tile last to shorten the tail
                pv_chunk(NC - 1, es_prev, [1, 2, 3, 0])
    nc.finalize()
    return nc


_NC_CACHE = {}


def _get_nc():
    if "nc" not in _NC_CACHE:
        _NC_CACHE["nc"] = _build_nc()
    return _NC_CACHE["nc"]


def _prep_inputs(x, W, b):
    # xt8[a2, p, i, t] = x[t, (2*a2+i)*128+p]
    xt = np.ascontiguousarray(
        x.reshape(B, T, NA, 2, P).transpose(0, 2, 4, 3, 1))       # [B,NA,P,2,T]
    xt8 = xt.astype(E4NP)
    # xtbf[p, a, t] = x[t<128, a*128+p]
    xtbf = np.ascontiguousarray(
        x[:, :P].reshape(B, P, NE, P).transpose(0, 3, 2, 1)).astype(BFNP)
    # wqk8[ft, p, a2, i, f] = 16*W[ft*128+f, (2*a2+i)*128+p]
    wqk8 = np.ascontiguousarray(
        (W[:2 * E] * WS).reshape(2 * NE, P, NA, 2, P).transpose(0, 4, 2, 3, 1)
    ).astype(E4NP)
    # wqkbf[ft, p, a, f] = W[ft*128+f, a*128+p]
    wqkbf = np.ascontiguousarray(
        W[:2 * E].reshape(2 * NE, P, NE, P).transpose(0, 3, 2, 1)).astype(BFNP)
    # wv8[a2, p, i, eo] = 16*W[2E+eo, (2*a2+i)*128+p]
    wv8 = np.ascontiguousarray(
        (W[2 * E:] * WS).reshape(E, NA, 2, P).transpose(1, 3, 2, 0)).astype(E4NP)
    # wvbf[p, a, eo] = 16*W[2E+eo, a*128+p]  (x16 so v0 matches v8 scaling)
    wvbf = np.ascontiguousarray(
        (W[2 * E:] * WS).reshape(E, NE, P).transpose(2, 1, 0)).astype(BFNP)
    # bqk[p, ft] = b[ft*128+p]
    bqk = np.ascontiguousarray(
        b[:2 * E].astype(np.float32).reshape(2 * NE, P).T).copy()
    bvrep = np.broadcast_to(b[2 * E:].astype(np.float32), (P, E)).copy()
    mska8 = (MA * np.eye(P, dtype=np.float32)).astype(E4NP)
    ii = np.arange(P)[:, None]
    jj = np.arange(CH)[None, :]
    # mskb8[r, j] = MB if j < r else 0 (within-tile triangular indicator)
    mskb8 = np.where(jj[:, :P] < ii, MB, 0.0).astype(np.float32).astype(E4NP)
    mskabf = (MA * np.eye(P, dtype=np.float32)).astype(BFNP)
    mskbbf = np.where(jj[:, :P] < ii, MB, 0.0).astype(np.float32).astype(BFNP)
    ones8 = np.full((P, 2, 4), WS, np.float32).astype(E4NP)
    ones0 = np.full((P, 4), WS, np.float32)
    shared = {"wqk8": wqk8, "wqkbf": wqkbf, "wv8": wv8, "wvbf": wvbf,
              "bqk": bqk, "bvrep": bvrep, "mska8": mska8, "mskb8": mskb8,
              "mskabf": mskabf, "mskbbf": mskbbf, "ones8": ones8,
              "ones0": ones0}
    return [{"xt8": np.ascontiguousarray(xt8[i]),
             "xtbf": np.ascontiguousarray(xtbf[i]), **shared}
            for i in range(B)]


def run(x, W, b, **spmd_kwargs):
    nc = _get_nc()
    in_maps = _prep_inputs(np.asarray(x, np.float32), np.asarray(W, np.float32),
                           np.asarray(b, np.float32))
    res = run_bass_kernel_spmd(nc, in_maps, list(range(B)), **spmd_kwargs)
    y = np.stack([res.results[i]["y"] for i in range(B)]).astype(np.float32)
    return y, res


def kernel(x, W, b):
    y, _ = run(x, W, b)
    return y
